# revision 18
# baseline (speedup 1.0000x reference)
"""Trainium2 Bass kernel for nn_BaselineSpanScorer (span-pair MLP scorer), v4.

reference:
    xs        [32, 512, 1024] f32
    spans     [65536, 2] int   (begin/end token index within sequence)
    batch_ids [65536] int
    W1 [2048, 150], b1 [150], W2 [150, 17], b2 [17]
    out[n] = relu(concat(xs[b, s0], xs[b, s1]) @ W1 + b1) @ W2 + b2

Strategy (8 NeuronCores, data parallel, per-core graph is input-shape fixed):
  - Shard xs by batch: core c owns batches [4c, 4c+4) = 2048 token rows.
  - Factorize: A[t] = xs[t] @ W1[:1024], G[t] = xs[t] @ W1[1024:], so
    pre[n] = A[i0_n] + G[i1_n] + b1 (token reuse cuts matmul work ~4x).
  - Stage 1: 16 token tiles x 8 K-blocks of fused [A|G] 300-col fp16
    matmuls, written into an SBUF token table tab_sb[128, 16 ranks, 512]:
    rank tt holds tokens tt*128+p, elems [0:150]=A row, [256:406]=G row.
  - Spans are bucketed by i0 token block (16 buckets of 512 + 1 overflow
    tile): a block tile's one-hot contraction needs exactly ONE 128-token
    window (A stays in SBUF as matmul lhsT). Overflow spans gather A.
  - G side (and overflow A side) via SBUF-source transpose dma_gather
    (features land on partitions; no DRAM round trip). All 18 gathers are
    PREPARED (descriptor-gen, ~1.2us each on Pool) during the xs load and
    fired by per-queue trigger_dma once the table's last write lands.
  - Stage 2 per span tile: PSUM accumulates onehot-A (or gathered A)
    + gathered G added on DVE; relu+bias on ACT; [150]x[17] W2
    contraction; bias; DMA out scores^T.
  - Host scatters per-core outputs back to original span order.

Compute dtype fp16 (rel err ~1e-3 vs f32 reference), f32 output.
"""

import os

os.environ.setdefault("MYCRO_LOCAL_CACHE", "1")

import numpy as np

# ---------------- problem constants (hardcoded per spec) ----------------
B, T, D = 32, 512, 1024
N_SPANS = 65536
H, L = 150, 17
NCORES = 8
BPC = B // NCORES        # batches per core = 4
TC = BPC * T             # tokens per core = 2048
N_KB = D // 128          # K blocks in stage 1 = 8
N_TT = TC // 128         # token tiles in stage 1 = 16 (= table ranks)
N_CH = 8                 # xsT load chunks (2 token tiles each)
SZ = 512                 # spans per stage-2 tile
NBT = TC // 128          # block tiles = 16
NT = NBT + 1             # + 1 overflow tile
MPAD = NT * SZ           # padded span count per core = 8704
HP2 = 256                # table row elems (fp16 -> 512B)
W1N = 2 * H              # 300: stage-1 moving operand width (A | G)
RANK_B = 2 * HP2 * 2     # table rank stride in bytes (A|G halves) = 1024


def build_graph():
    """Build the per-core SPMD Bass graph (fixed shapes, input independent)."""
    from concourse import bacc
    import concourse.mybir as mybir
    from concourse.tile import TileContext
    from concourse import library_config
    from concourse.tile_rust import add_dep_helper

    fp16 = mybir.dt.float16
    f32 = mybir.dt.float32
    i16 = mybir.dt.int16
    u8 = mybir.dt.uint8
    AF = mybir.ActivationFunctionType
    EQ = mybir.AluOpType.is_equal
    ADD = mybir.AluOpType.add

    nc = bacc.Bacc(num_swdge_queues=4)

    from concourse.tile_rust import add_dep_helper as _adh
    _chains = {"sync": []}

    def _chained(which, eng):
        def dma(**kw):
            g = eng(**kw)
            if _chains[which]:
                _adh(g.ins, _chains[which][-1].ins, False, "pin dma order")
            _chains[which].append(g)
            return g
        return dma

    xsT_d = nc.declare_dram_parameter("xsT", [128, N_CH, N_KB, 256], fp16, isOutput=False)
    wc_d = nc.declare_dram_parameter("wc", [128, N_KB * W1N], fp16, isOutput=False)
    w2p_d = nc.declare_dram_parameter("w2p", [128, 2 * L], fp16, isOutput=False)
    b1p_d = nc.declare_dram_parameter("b1p", [128, 2], f32, isOutput=False)
    b2p_d = nc.declare_dram_parameter("b2p", [L, 1], f32, isOutput=False)
    iota_d = nc.declare_dram_parameter("iota", [128, 1], u8, isOutput=False)
    i0v_d = nc.declare_dram_parameter("i0v", [1, NBT * SZ], u8, isOutput=False)
    idxg_d = nc.declare_dram_parameter("idxg", [128, NT * 32], i16, isOutput=False)
    idxa_d = nc.declare_dram_parameter("idxa", [128, 32], i16, isOutput=False)
    outT_d = nc.declare_dram_parameter("outT", [L, MPAD], f32, isOutput=True)

    sync_dma = _chained("sync", nc.sync.dma_start)

    with TileContext(nc) as tc:
        with (
            tc.tile_pool(name="const", bufs=1) as constp,
            tc.tile_pool(name="xst", bufs=1) as xstp,
            tc.tile_pool(name="tab", bufs=1) as tabp,
            tc.tile_pool(name="s0", bufs=1) as s0p,
            tc.tile_pool(name="gg", bufs=1) as ggp,
            tc.tile_pool(name="psX", bufs=5, space="PSUM") as psXp,
            tc.tile_pool(name="ps2", bufs=3, space="PSUM") as ps2p,
            tc.tile_pool(name="h0", bufs=6) as h0p,
            tc.tile_pool(name="t0", bufs=6) as t0p,
            tc.tile_pool(name="t1", bufs=6) as t1p,
            tc.tile_pool(name="h1", bufs=6) as h1p,
            tc.tile_pool(name="ot", bufs=8) as otp,
        ):
            nc.gpsimd.load_library(library_config.mlp)

            # ---- input loads ----
            # stage-1 weights first (PE needs them at t~1us), then the small
            # gather-index / one-hot tensors (Pool engine prep work needs
            # them early), then the bulk xs^T chunks. The tile scheduler
            # reorders DMA issues, so pin queue order explicitly.
            wc_lo = constp.tile([128, 4 * W1N], fp16)
            wc_hi = constp.tile([128, 4 * W1N], fp16)
            wc_dma = sync_dma(out=wc_lo[:], in_=wc_d[:, 0:4 * W1N])
            idxg_sb = constp.tile([128, NT * 32], i16)
            sync_dma(out=idxg_sb[:], in_=idxg_d[:])
            idxa_sb = constp.tile([128, 32], i16)
            sync_dma(out=idxa_sb[:], in_=idxa_d[:])
            iota_sb = constp.tile([128, 1], u8)
            sync_dma(out=iota_sb[:], in_=iota_d[:])
            i0v_sm = constp.tile([1, NBT * SZ], u8)
            sync_dma(out=i0v_sm[:], in_=i0v_d[:])
            sync_dma(out=wc_hi[:], in_=wc_d[:, 4 * W1N:])
            # xs^T chunks on the sync queue
            xst_sb = xstp.tile([128, N_CH, N_KB, 256], fp16)
            xst_dmas = [wc_dma]
            for ch in range(N_CH):
                x = sync_dma(
                    out=xst_sb[:, ch, :, :], in_=xsT_d[:, ch, :, :]
                )
                add_dep_helper(x.ins, xst_dmas[-1].ins, False, "chunk order")
                xst_dmas.append(x)
            w2p_sb = constp.tile([128, 2 * L], fp16)
            c1 = sync_dma(out=w2p_sb[:], in_=w2p_d[:])
            b1p_sb = constp.tile([128, 2], f32)
            c2 = sync_dma(out=b1p_sb[:], in_=b1p_d[:])
            b2p_sb = constp.tile([L, 1], f32)
            c3 = sync_dma(out=b2p_sb[:], in_=b2p_d[:])
            for c in (c1, c2, c3):
                add_dep_helper(c.ins, xst_dmas[1].ins, False, "xst loads first")

            # ---- broadcast the one-hot compare row to 128 partitions ----
            i0v_sb = constp.tile([128, NBT * SZ], u8)
            bcast = nc.gpsimd.partition_broadcast(i0v_sb[:], i0v_sm[:])

            # ---- SBUF token table: rank tt = tokens tt*128+p;
            # fp16 elems [0:150] = A row, [256:406] = G row, pads zeroed ----
            tab_sb = tabp.tile([128, N_TT, 2 * HP2], fp16)

            # ---- A-side one-hot tiles (DVE; fully overlapped with stage 1) ----
            s0_tiles = []
            for b in range(NBT):
                s0 = s0p.tile([128, SZ], fp16, tag=f"s0_{b}", name=f"s0_{b}")
                nc.vector.tensor_tensor(
                    out=s0[:],
                    in0=iota_sb[:, 0:1].to_broadcast([128, SZ]),
                    in1=i0v_sb[:, b * SZ:(b + 1) * SZ],
                    op=EQ,
                )
                s0_tiles.append(s0)

            # ---- stage 1: xs @ [W1a | W1g] -> SBUF token table ----
            tab_writers = []
            for tt in range(N_TT):
                ch, hf = tt // 2, tt % 2
                ps = psXp.tile([128, 512], f32, tag="ps")
                for kb in range(N_KB):
                    wch = wc_lo if kb < 4 else wc_hi
                    nc.tensor.matmul(
                        ps[:, 0:W1N],
                        xst_sb[:, ch, kb, hf * 128:(hf + 1) * 128],
                        wch[:, (kb % 4) * W1N:(kb % 4 + 1) * W1N],
                        start=(kb == 0),
                        stop=(kb == N_KB - 1),
                    )
                tab_writers.append(nc.vector.memset(tab_sb[:, tt, H:HP2], 0.0))
                tab_writers.append(
                    nc.vector.memset(tab_sb[:, tt, HP2 + H:2 * HP2], 0.0)
                )
                tab_writers.append(
                    nc.scalar.activation(tab_sb[:, tt, 0:H], ps[:, 0:H], AF.Copy)
                )
                tab_writers.append(
                    nc.scalar.activation(
                        tab_sb[:, tt, HP2:HP2 + H], ps[:, H:W1N], AF.Copy
                    )
                )
            tw_by_name = {w.ins.name: w.ins for w in tab_writers}

            # ---- gather preps: descriptor-gen on Pool engine. Emitted
            # AFTER the table writes so the RAW edge on tab_sb is deferred
            # to the trigger (the preps themselves keep only no-sync edges
            # and execute early, overlapped with the xs load); the DMAs
            # fire via per-queue trigger_dma once the table is complete.
            # Chain preps so scheduled order matches emission order (Tile
            # locks DMASW lane sems to SWDGE queues). 512 idxs per gather:
            # >512-idx transpose gathers overflow the 128-entry SWDGE ring.
            gg_sb = ggp.tile([128, NT + 1, 2, SZ], fp16)
            # one completion sem per consumer tile: G gather st -> gsems[st]
            # (wait >= 16); the overflow tile's two gathers share gsems[16]
            # (wait >= 32 -- a stable value for the race detector, no
            # cross-gather ordering assumption on HW).
            gsems = [nc.alloc_semaphore(f"ggs{k}") for k in range(NT)]
            prev_g = bcast
            qn = 0

            def emit_prep(out_ap, idx_ap, byte_off, sem, queue=None):
                nonlocal prev_g, qn
                q = qn % 4 if queue is None else queue
                g = nc.gpsimd.dma_gather(
                    out_ap,
                    tab_sb[:],
                    idx_ap,
                    SZ,
                    SZ,
                    elem_size=HP2,
                    transpose=True,
                    queue_num=q,
                    prepare_only=True,
                    sem=sem,
                    sbuf_tokens_per_rank=128,
                    sbuf_free_dim_per_rank=RANK_B,
                    sbuf_byte_offset=byte_off,
                )
                # Tile does not defer a gather's SBUF-source RAW to the
                # trigger (it does for scatter_add) — do it by hand: demote
                # the prep's sync deps on the table writers to no-sync
                # (keeps topo order, drops the semaphore wait) so desc-gen
                # overlaps the xs load; the triggers get the sync deps.
                for dep in set(g.ins.sync_dependency_names()):
                    if dep in tw_by_name:
                        g.ins.try_remove_dependency(dep)
                        add_dep_helper(g.ins, tw_by_name[dep], False, "defer src")
                add_dep_helper(g.ins, prev_g.ins, False, "pin prep order")
                prev_g = g
                qn += 1
                gq.append(q)
                return g

            preps = []
            gq = []          # gather k -> queue
            for st in range(NT):
                # G rows for span tile st (begin-block bucketed, i1 arbitrary)
                preps.append(emit_prep(
                    gg_sb[:, st, :, :],
                    idxg_sb[:, st * 32:(st + 1) * 32],
                    2 * HP2,
                    gsems[st],
                ))
            # A rows for the overflow tile: shares gsems[16] with its G
            # gather, so it must ride the same SWDGE queue (sem locking)
            preps.append(
                emit_prep(gg_sb[:, NT, :, :], idxa_sb[:], 0, gsems[16], queue=0)
            )
            prep_by_name = {p.ins.name: p.ins for p in preps}

            # ---- fire the prepared gathers (table complete) ----
            # One count=1 trigger per gather, in gather order: each queue
            # drains FIFO stepwise (well-defined per-queue sem values for
            # the consumers). Only the first trigger needs the table sync
            # deps — the Pool engine executes triggers serially after it.
            for k in range(len(preps)):
                trig = nc.gpsimd.trigger_dma(count=1, queue_num=gq[k])
                add_dep_helper(trig.ins, preps[k].ins, False, "after prep")
                if k == 0:
                    for w in tab_writers:
                        add_dep_helper(trig.ins, w.ins, True, "table -> trigger")

            def gg_lo(st):
                return gg_sb[:, st, 0, :]

            def gg_hi(st):
                return gg_sb[0:22, st, 1, :]

            def degate(inst, *gathers):
                """Replace Tile's prep-write attribution (whose DMASW lane
                sem is pre-bumped, not DMA-completion-gated, in prep mode)
                with waits fused onto the consumer: each gather's completion
                bumps its queue's sem by 16, in FIFO order per queue."""
                for dep in set(inst.ins.sync_dependency_names()):
                    if dep in prep_by_name:
                        inst.ins.try_remove_dependency(dep)
                        add_dep_helper(
                            inst.ins, prep_by_name[dep], False, "manual qsem"
                        )
                sems = {}
                for k in gathers:
                    s = min(k, NT - 1)
                    sems[s] = sems.get(s, 0) + 16
                for s, val in sems.items():
                    inst.wait_op(gsems[s], val, "sem-ge")

            # ---- stage 2 ----
            for st in range(NT):
                t0 = t0p.tile([128, SZ], fp16, tag="t0")
                t1 = t1p.tile([22, SZ], fp16, tag="t1")
                if st < NBT:
                    pA0 = psXp.tile([128, SZ], f32, tag="ps")
                    nc.tensor.matmul(
                        pA0[:], tab_sb[:, st, 0:128], s0_tiles[st][:],
                        start=True, stop=True,
                    )
                    a0 = nc.vector.tensor_tensor(
                        out=t0[:], in0=pA0[:], in1=gg_lo(st), op=ADD
                    )
                    pA1 = psXp.tile([22, SZ], f32, tag="ps")
                    nc.tensor.matmul(
                        pA1[:], tab_sb[:, st, 128:H], s0_tiles[st][:],
                        start=True, stop=True,
                    )
                    a1 = nc.vector.tensor_tensor(
                        out=t1[:], in0=pA1[:], in1=gg_hi(st), op=ADD
                    )
                    degate(a0, st)
                    degate(a1, st)
                else:
                    a0 = nc.vector.tensor_tensor(
                        out=t0[:], in0=gg_sb[:, NT, 0, :], in1=gg_lo(st), op=ADD
                    )
                    a1 = nc.vector.tensor_tensor(
                        out=t1[:], in0=gg_sb[0:22, NT, 1, :], in1=gg_hi(st), op=ADD
                    )
                    degate(a0, st, st + 1)
                    degate(a1, st, st + 1)  # both -> gsems[16] >= 32
                h0 = h0p.tile([128, SZ], fp16, tag="h0")
                h1 = h1p.tile([22, SZ], fp16, tag="h1")
                nc.scalar.activation(h0[:], t0[:], AF.Relu, bias=b1p_sb[:, 0:1])
                nc.scalar.activation(h1[:], t1[:], AF.Relu, bias=b1p_sb[0:22, 1:2])
                ps2 = ps2p.tile([L, SZ], f32, tag="ps2")
                nc.tensor.matmul(
                    ps2[:], w2p_sb[:, 0:L], h0[:], start=True, stop=False
                )
                nc.tensor.matmul(
                    ps2[:], w2p_sb[0:22, L:2 * L], h1[:], start=False, stop=True
                )
                ot = otp.tile([L, SZ], f32)
                if st % 2 == 0:
                    nc.scalar.activation(ot[:], ps2[:], AF.Identity, bias=b2p_sb[:])
                else:
                    nc.vector.tensor_scalar_add(out=ot[:], in0=ps2[:], scalar1=b2p_sb[:])
                sync_dma(
                    out=outT_d[:, st * SZ:(st + 1) * SZ], in_=ot[:]
                )

    return nc


def _wrap_idx(idx_1d):
    """SWDGE index layout: [16, n/16].T wrapped, replicated to 128 rows."""
    n = len(idx_1d)
    arr16 = idx_1d.reshape(n // 16, 16).T
    return np.tile(arr16, (8, 1)).astype(np.int16)


def prep_inputs(xs, spans, batch_ids, W1, b1, W2, b2):
    """Host-side routing and layout. Returns (in_maps, slot_maps)."""
    xs = np.asarray(xs, dtype=np.float32)
    spans = np.asarray(spans).astype(np.int64)
    batch_ids = np.asarray(batch_ids).astype(np.int64)
    W1 = np.asarray(W1, dtype=np.float32)
    b1 = np.asarray(b1, dtype=np.float32)
    W2 = np.asarray(W2, dtype=np.float32)
    b2 = np.asarray(b2, dtype=np.float32)

    core = batch_ids // BPC
    local0 = (batch_ids % BPC) * T + spans[:, 0]
    local1 = (batch_ids % BPC) * T + spans[:, 1]

    # shared weights
    W1h = W1.astype(np.float16)
    wc = np.empty((128, N_KB * W1N), np.float16)
    for kb in range(N_KB):
        wc[:, kb * W1N:kb * W1N + H] = W1h[kb * 128:(kb + 1) * 128, :]
        wc[:, kb * W1N + H:(kb + 1) * W1N] = W1h[D + kb * 128:D + (kb + 1) * 128, :]
    w2p = np.zeros((128, 2 * L), np.float16)
    w2p[:, 0:L] = W2[0:128].astype(np.float16)
    w2p[0:H - 128, L:2 * L] = W2[128:H].astype(np.float16)
    b1p = np.zeros((128, 2), np.float32)
    b1p[:, 0] = b1[0:128]
    b1p[0:H - 128, 1] = b1[128:H]
    b2p = np.ascontiguousarray(b2.reshape(L, 1))
    iota = np.arange(128, dtype=np.uint8).reshape(128, 1)

    in_maps = []
    slot_maps = []
    for c in range(NCORES):
        sel = np.nonzero(core == c)[0]
        i0 = local0[sel]
        i1 = local1[sel]
        blk = i0 >> 7

        slot_map = np.full(MPAD, -1, np.int64)
        s_i0 = np.zeros(MPAD, np.int64)
        s_i1 = np.zeros(MPAD, np.int64)
        overflow = []
        for b in range(NBT):
            ids = np.nonzero(blk == b)[0]
            take = ids[:SZ]
            overflow.append(ids[SZ:])
            base = b * SZ
            ntk = len(take)
            slot_map[base:base + ntk] = sel[take]
            s_i0[base:base + ntk] = i0[take]
            s_i1[base:base + ntk] = i1[take]
            # pad: i0 at block start (one-hot row 0), i1 = 0
            s_i0[base + ntk:base + SZ] = b * 128
        ov = np.concatenate(overflow) if overflow else np.empty(0, np.int64)
        assert len(ov) <= SZ, f"core {c}: overflow {len(ov)} > {SZ}"
        base = NBT * SZ
        nov = len(ov)
        slot_map[base:base + nov] = sel[ov]
        s_i0[base:base + nov] = i0[ov]
        s_i1[base:base + nov] = i1[ov]

        # one-hot compare values: block-relative begin-token, u8, single row
        i0v = np.ascontiguousarray(
            (s_i0[:NBT * SZ] & 127).astype(np.uint8).reshape(1, NBT * SZ)
        )
        idxg = _wrap_idx(s_i1)
        idxa = _wrap_idx(s_i0[NBT * SZ:])

        # xsT: [p, ch, kb, t] = xs[token=ch*256+t, d=kb*128+p], fp16
        xs_c = xs[c * BPC:(c + 1) * BPC].reshape(TC, D).astype(np.float16)
        xsT = np.ascontiguousarray(
            xs_c.T.reshape(N_KB, 128, N_CH, 256).transpose(1, 2, 0, 3)
        )

        in_maps.append({
            "xsT": xsT, "wc": wc, "w2p": w2p,
            "b1p": b1p, "b2p": b2p, "iota": iota, "i0v": i0v,
            "idxg": idxg, "idxa": idxa,
        })
        slot_maps.append(slot_map)

    return in_maps, slot_maps


def _scatter_out(results, slot_maps):
    out = np.empty((N_SPANS, L), np.float32)
    for c in range(NCORES):
        sm = slot_maps[c]
        valid = sm >= 0
        out[sm[valid]] = results[c]["outT"].T[valid]
    return out


def _install_ntff_shim():
    """Provide antenv.axon_hooks (missing on this image) so that
    run_bass_kernel_spmd(trace=True) can drive NTFF profiling via the
    axon .so. Only used by the profiling path."""
    import sys
    import types
    import ctypes
    import contextlib

    if "antenv.axon_hooks" in sys.modules:
        return
    import antenv

    holder = {"hook": None}
    mod = types.ModuleType("antenv.axon_hooks")
    mod.set_axon_ntff_profile_hook = lambda h: holder.__setitem__("hook", h)
    mod.get_axon_ntff_profile_hook = lambda: holder["hook"]
    sys.modules["antenv.axon_hooks"] = mod
    antenv.axon_hooks = mod

    so_path = "/opt/axon/libaxon_pjrt.so"
    try:
        lib = ctypes.CDLL(so_path)
    except OSError:
        return
    if not hasattr(lib, "axon_start_nrt_profile"):
        return
    lib.axon_start_nrt_profile.argtypes = [
        ctypes.POINTER(ctypes.c_int64),
        ctypes.c_size_t,
    ]
    lib.axon_start_nrt_profile.restype = ctypes.c_int64
    lib.axon_stop_nrt_profile.argtypes = [ctypes.c_char_p]
    lib.axon_stop_nrt_profile.restype = ctypes.c_int64

    @contextlib.contextmanager
    def _hook(output_dir, device_ids):
        import jax

        jax.devices()
        if device_ids:
            ids = (ctypes.c_int64 * len(device_ids))(*device_ids)
            rc = lib.axon_start_nrt_profile(ids, len(device_ids))
        else:
            rc = lib.axon_start_nrt_profile(None, 0)
        if rc != 0:
            raise RuntimeError(f"axon_start_nrt_profile rc={rc}")
        try:
            yield
        finally:
            n = lib.axon_stop_nrt_profile(str(output_dir).encode())
            print(f"profile: {n} file(s) written to {output_dir}")

    mod.set_axon_ntff_profile_hook(_hook)


def run(inputs: dict, trace: bool = False):
    """Run on the 8 NeuronCores. Returns (out, BassKernelResults)."""
    from concourse import bass_utils
    from concourse.bass_utils import run_bass_kernel_spmd

    if trace:
        _install_ntff_shim()
        bass_utils.upload_artifacts = lambda tmpdir: str(tmpdir)

    in_maps, slot_maps = prep_inputs(**inputs)
    nc = build_graph()
    nc.finalize()
    res = run_bass_kernel_spmd(
        nc, in_maps, list(range(NCORES)), trace=trace
    )
    return _scatter_out(res.results, slot_maps), res


def kernel(**inputs) -> np.ndarray:
    out, _ = run(inputs, trace=False)
    return out


# revision 21
# speedup vs baseline: 1.1230x; 1.1230x over previous
"""Trainium2 Bass kernel for nn_BaselineSpanScorer (span-pair MLP scorer), v4.

reference:
    xs        [32, 512, 1024] f32
    spans     [65536, 2] int   (begin/end token index within sequence)
    batch_ids [65536] int
    W1 [2048, 150], b1 [150], W2 [150, 17], b2 [17]
    out[n] = relu(concat(xs[b, s0], xs[b, s1]) @ W1 + b1) @ W2 + b2

Strategy (8 NeuronCores, data parallel, per-core graph is input-shape fixed):
  - Shard xs by batch: core c owns batches [4c, 4c+4) = 2048 token rows.
  - Factorize: A[t] = xs[t] @ W1[:1024], G[t] = xs[t] @ W1[1024:], so
    pre[n] = A[i0_n] + G[i1_n] + b1 (token reuse cuts matmul work ~4x).
  - Stage 1: 16 token tiles x 8 K-blocks of fused [A|G] 300-col fp16
    matmuls. A tiles stay in SBUF (one-hot lhsT) and are also DMAed to a
    DRAM table (for the gathers); G tiles go to the same DRAM table.
  - Spans are bucketed by i0 token block (16 buckets of 512 + 1 overflow
    tile): a block tile's one-hot contraction needs exactly ONE 128-token
    window (A stays in SBUF as matmul lhsT). Overflow spans gather A.
  - G side (and overflow A side) via DRAM-source transpose dma_gather
    (features land on partitions). All 18 gathers are PREPARED
    (descriptor-gen, ~1us each on Pool) during the xs load and fired by
    per-gather trigger_dma once the table's last DRAM write lands.
    (SBUF-source gathers measured 2.4x slower transfer; Tile's prep-mode
    DMASW pre-bump insts are stripped post-build -- nothing waits on them,
    consumers are gated by the per-gather descriptor sems instead.)
  - Stage 2 per span tile: PSUM accumulates onehot-A (or gathered A)
    + gathered G added on DVE; relu+bias on ACT; [150]x[17] W2
    contraction; bias; DMA out scores^T.
  - Host scatters per-core outputs back to original span order.

Compute dtype fp16 (rel err ~1e-3 vs f32 reference), f32 output.
"""

import os

os.environ.setdefault("MYCRO_LOCAL_CACHE", "1")

import numpy as np

# ---------------- problem constants (hardcoded per spec) ----------------
B, T, D = 32, 512, 1024
N_SPANS = 65536
H, L = 150, 17
NCORES = 8
BPC = B // NCORES        # batches per core = 4
TC = BPC * T             # tokens per core = 2048
N_KB = D // 128          # K blocks in stage 1 = 8
N_TT = TC // 128         # token tiles in stage 1 = 16 (= table ranks)
N_CH = 8                 # xsT load chunks (2 token tiles each)
SZ = 512                 # spans per stage-2 tile
NBT = TC // 128          # block tiles = 16
NT = NBT + 1             # + 1 overflow tile
MPAD = NT * SZ           # padded span count per core = 8704
HP2 = 256                # table row elems (fp16 -> 512B)
W1N = 2 * H              # 300: stage-1 moving operand width (A | G)
RANK_B = 2 * HP2 * 2     # table rank stride in bytes (A|G halves) = 1024


def build_graph():
    """Build the per-core SPMD Bass graph (fixed shapes, input independent)."""
    from concourse import bacc
    import concourse.mybir as mybir
    from concourse.tile import TileContext
    from concourse import library_config
    from concourse.tile_rust import add_dep_helper

    fp16 = mybir.dt.float16
    f32 = mybir.dt.float32
    i16 = mybir.dt.int16
    u8 = mybir.dt.uint8
    AF = mybir.ActivationFunctionType
    EQ = mybir.AluOpType.is_equal
    ADD = mybir.AluOpType.add

    nc = bacc.Bacc(num_swdge_queues=4)

    from concourse.tile_rust import add_dep_helper as _adh
    _chains = {"sync": []}

    def _chained(which, eng):
        def dma(**kw):
            g = eng(**kw)
            if _chains[which]:
                _adh(g.ins, _chains[which][-1].ins, False, "pin dma order")
            _chains[which].append(g)
            return g
        return dma

    xsT_d = nc.declare_dram_parameter("xsT", [128, N_CH, N_KB, 256], fp16, isOutput=False)
    wc_d = nc.declare_dram_parameter("wc", [128, N_KB * W1N], fp16, isOutput=False)
    w2p_d = nc.declare_dram_parameter("w2p", [128, 2 * L], fp16, isOutput=False)
    b1p_d = nc.declare_dram_parameter("b1p", [128, 2], f32, isOutput=False)
    b2p_d = nc.declare_dram_parameter("b2p", [L, 1], f32, isOutput=False)
    iota_d = nc.declare_dram_parameter("iota", [128, 1], u8, isOutput=False)
    i0v_d = nc.declare_dram_parameter("i0v", [1, NBT * SZ], u8, isOutput=False)
    idxg_d = nc.declare_dram_parameter("idxg", [128, NT * 32], i16, isOutput=False)
    idxa_d = nc.declare_dram_parameter("idxa", [128, 32], i16, isOutput=False)
    outT_d = nc.declare_dram_parameter("outT", [L, MPAD], f32, isOutput=True)

    sync_dma = _chained("sync", nc.sync.dma_start)

    with TileContext(nc) as tc:
        with (
            tc.tile_pool(name="const", bufs=1) as constp,
            tc.tile_pool(name="xst", bufs=1) as xstp,
            tc.tile_pool(name="atile", bufs=1) as atilep,
            tc.tile_pool(name="dram", bufs=1, space="DRAM") as dramp,
            tc.tile_pool(name="s0", bufs=1) as s0p,
            tc.tile_pool(name="gg", bufs=1) as ggp,
            tc.tile_pool(name="psX", bufs=5, space="PSUM") as psXp,
            tc.tile_pool(name="ps2", bufs=3, space="PSUM") as ps2p,
            tc.tile_pool(name="h0", bufs=6) as h0p,
            tc.tile_pool(name="t0", bufs=6) as t0p,
            tc.tile_pool(name="t1", bufs=6) as t1p,
            tc.tile_pool(name="h1", bufs=6) as h1p,
            tc.tile_pool(name="ot", bufs=8) as otp,
        ):
            nc.gpsimd.load_library(library_config.mlp)

            # ---- input loads ----
            # stage-1 weights first (PE needs them at t~1us), then the small
            # gather-index / one-hot tensors (Pool engine prep work needs
            # them early), then the bulk xs^T chunks. The tile scheduler
            # reorders DMA issues, so pin queue order explicitly.
            wc_lo = constp.tile([128, 4 * W1N], fp16)
            wc_hi = constp.tile([128, 4 * W1N], fp16)
            wc_dma = sync_dma(out=wc_lo[:], in_=wc_d[:, 0:4 * W1N])
            idxg_sb = constp.tile([128, NT * 32], i16)
            sync_dma(out=idxg_sb[:], in_=idxg_d[:])
            idxa_sb = constp.tile([128, 32], i16)
            sync_dma(out=idxa_sb[:], in_=idxa_d[:])
            iota_sb = constp.tile([128, 1], u8)
            sync_dma(out=iota_sb[:], in_=iota_d[:])
            i0v_sm = constp.tile([1, NBT * SZ], u8)
            sync_dma(out=i0v_sm[:], in_=i0v_d[:])
            sync_dma(out=wc_hi[:], in_=wc_d[:, 4 * W1N:])
            # xs^T chunks on the sync queue
            xst_sb = xstp.tile([128, N_CH, N_KB, 256], fp16)
            xst_dmas = [wc_dma]
            for ch in range(N_CH):
                x = sync_dma(
                    out=xst_sb[:, ch, :, :], in_=xsT_d[:, ch, :, :]
                )
                add_dep_helper(x.ins, xst_dmas[-1].ins, False, "chunk order")
                xst_dmas.append(x)
            w2p_sb = constp.tile([128, 2 * L], fp16)
            c1 = sync_dma(out=w2p_sb[:], in_=w2p_d[:])
            b1p_sb = constp.tile([128, 2], f32)
            c2 = sync_dma(out=b1p_sb[:], in_=b1p_d[:])
            b2p_sb = constp.tile([L, 1], f32)
            c3 = sync_dma(out=b2p_sb[:], in_=b2p_d[:])
            for c in (c1, c2, c3):
                add_dep_helper(c.ins, xst_dmas[1].ins, False, "xst loads first")

            # ---- broadcast the one-hot compare row to 128 partitions ----
            i0v_sb = constp.tile([128, NBT * SZ], u8)
            bcast = nc.gpsimd.partition_broadcast(i0v_sb[:], i0v_sm[:])

            # ---- combined DRAM gather table: token t -> [A row | G row],
            # 512B each; gathers pick a half via base offset + elem_step ----
            tab = dramp.tile([TC, 2, HP2], fp16)

            # ---- A-side one-hot tiles (DVE; fully overlapped with stage 1) ----
            s0_tiles = []
            for b in range(NBT):
                s0 = s0p.tile([128, SZ], fp16, tag=f"s0_{b}", name=f"s0_{b}")
                nc.vector.tensor_tensor(
                    out=s0[:],
                    in0=iota_sb[:, 0:1].to_broadcast([128, SZ]),
                    in1=i0v_sb[:, b * SZ:(b + 1) * SZ],
                    op=EQ,
                )
                s0_tiles.append(s0)

            # ---- stage 1: xs @ [W1a | W1g] -> A tiles in SBUF + DRAM table ----
            a_tiles = []
            tab_writers = []
            last_tab_dma = None
            for tt in range(N_TT):
                ch, hf = tt // 2, tt % 2
                ps = psXp.tile([128, 512], f32, tag="ps")
                for kb in range(N_KB):
                    wch = wc_lo if kb < 4 else wc_hi
                    nc.tensor.matmul(
                        ps[:, 0:W1N],
                        xst_sb[:, ch, kb, hf * 128:(hf + 1) * 128],
                        wch[:, (kb % 4) * W1N:(kb % 4 + 1) * W1N],
                        start=(kb == 0),
                        stop=(kb == N_KB - 1),
                    )
                at = atilep.tile([128, 2, HP2], fp16, tag=f"a_{tt}", name=f"a_{tt}")
                nc.vector.memset(at[:, 0, H:HP2], 0.0)
                nc.vector.memset(at[:, 1, H:HP2], 0.0)
                nc.scalar.activation(at[:, 0, 0:H], ps[:, 0:H], AF.Copy)
                nc.scalar.activation(at[:, 1, 0:H], ps[:, H:W1N], AF.Copy)
                # same-queue FIFO: the last table write covers all 16
                last_tab_dma = sync_dma(
                    out=tab[tt * 128:(tt + 1) * 128, :, :], in_=at[:]
                )
                tab_writers.append(last_tab_dma)
                a_tiles.append(at)
            tw_by_name = {w.ins.name: w.ins for w in tab_writers}

            # ---- gather preps: descriptor-gen on Pool engine. Emitted
            # AFTER the table writes so the RAW edge on tab_sb is deferred
            # to the trigger (the preps themselves keep only no-sync edges
            # and execute early, overlapped with the xs load); the DMAs
            # fire via per-queue trigger_dma once the table is complete.
            # Chain preps so scheduled order matches emission order (Tile
            # locks DMASW lane sems to SWDGE queues). 512 idxs per gather:
            # >512-idx transpose gathers overflow the 128-entry SWDGE ring.
            gg_sb = ggp.tile([128, NT + 1, 2, SZ], fp16)
            # one completion sem per consumer tile: G gather st -> gsems[st]
            # (wait >= 16); the overflow tile's two gathers share gsems[16]
            # (wait >= 32 -- a stable value for the race detector, no
            # cross-gather ordering assumption on HW).
            gsems = [nc.alloc_semaphore(f"ggs{k}") for k in range(NT)]
            prev_g = bcast
            qn = 0

            def emit_prep(out_ap, idx_ap, half, sem, queue=None):
                nonlocal prev_g, qn
                q = qn % 4 if queue is None else queue
                g = nc.gpsimd.dma_gather(
                    out_ap,
                    tab[:, half, :],
                    idx_ap,
                    SZ,
                    SZ,
                    elem_size=HP2,
                    elem_step=2 * HP2,
                    transpose=True,
                    queue_num=q,
                    prepare_only=True,
                    sem=sem,
                )
                # Tile does not defer a gather's SBUF-source RAW to the
                # trigger (it does for scatter_add) — do it by hand: demote
                # the prep's sync deps on the table writers to no-sync
                # (keeps topo order, drops the semaphore wait) so desc-gen
                # overlaps the xs load; the triggers get the sync deps.
                for dep in set(g.ins.sync_dependency_names()):
                    if dep in tw_by_name:
                        g.ins.try_remove_dependency(dep)
                        add_dep_helper(g.ins, tw_by_name[dep], False, "defer src")
                add_dep_helper(g.ins, prev_g.ins, False, "pin prep order")
                prev_g = g
                qn += 1
                gq.append(q)
                return g

            preps = []
            gq = []          # gather k -> queue
            for st in range(NT):
                # G rows for span tile st (begin-block bucketed, i1 arbitrary)
                preps.append(emit_prep(
                    gg_sb[:, st, :, :],
                    idxg_sb[:, st * 32:(st + 1) * 32],
                    1,
                    gsems[st],
                ))
            # A rows for the overflow tile: shares gsems[16] with its G
            # gather, so it must ride the same SWDGE queue (sem locking)
            preps.append(
                emit_prep(gg_sb[:, NT, :, :], idxa_sb[:], 0, gsems[16], queue=0)
            )  # half=0: A rows
            prep_by_name = {p.ins.name: p.ins for p in preps}

            # ---- fire the prepared gathers (table complete) ----
            # One count=1 trigger per gather, in gather order: each queue
            # drains FIFO stepwise (well-defined per-queue sem values for
            # the consumers). Only the first trigger needs the table sync
            # deps — the Pool engine executes triggers serially after it.
            for k in range(len(preps)):
                trig = nc.gpsimd.trigger_dma(count=1, queue_num=gq[k])
                add_dep_helper(trig.ins, preps[k].ins, False, "after prep")
                if k == 0:
                    add_dep_helper(
                        trig.ins, last_tab_dma.ins, True, "table -> trigger"
                    )

            def gg_lo(st):
                return gg_sb[:, st, 0, :]

            def gg_hi(st):
                return gg_sb[0:22, st, 1, :]

            def degate(inst, *gathers):
                """Replace Tile's prep-write attribution (whose DMASW lane
                sem is pre-bumped, not DMA-completion-gated, in prep mode)
                with waits fused onto the consumer: each gather's completion
                bumps its queue's sem by 16, in FIFO order per queue."""
                for dep in set(inst.ins.sync_dependency_names()):
                    if dep in prep_by_name:
                        inst.ins.try_remove_dependency(dep)
                        add_dep_helper(
                            inst.ins, prep_by_name[dep], False, "manual qsem"
                        )
                sems = {}
                for k in gathers:
                    s = min(k, NT - 1)
                    sems[s] = sems.get(s, 0) + 16
                for s, val in sems.items():
                    inst.wait_op(gsems[s], val, "sem-ge")

            # ---- stage 2 ----
            for st in range(NT):
                t0 = t0p.tile([128, SZ], fp16, tag="t0")
                t1 = t1p.tile([22, SZ], fp16, tag="t1")
                if st < NBT:
                    pA0 = psXp.tile([128, SZ], f32, tag="ps")
                    nc.tensor.matmul(
                        pA0[:], a_tiles[st][:, 0, 0:128], s0_tiles[st][:],
                        start=True, stop=True,
                    )
                    a0 = nc.vector.tensor_tensor(
                        out=t0[:], in0=pA0[:], in1=gg_lo(st), op=ADD
                    )
                    pA1 = psXp.tile([22, SZ], f32, tag="ps")
                    nc.tensor.matmul(
                        pA1[:], a_tiles[st][:, 0, 128:H], s0_tiles[st][:],
                        start=True, stop=True,
                    )
                    a1 = nc.vector.tensor_tensor(
                        out=t1[:], in0=pA1[:], in1=gg_hi(st), op=ADD
                    )
                    degate(a0, st)
                    degate(a1, st)
                else:
                    a0 = nc.vector.tensor_tensor(
                        out=t0[:], in0=gg_sb[:, NT, 0, :], in1=gg_lo(st), op=ADD
                    )
                    a1 = nc.vector.tensor_tensor(
                        out=t1[:], in0=gg_sb[0:22, NT, 1, :], in1=gg_hi(st), op=ADD
                    )
                    degate(a0, st, st + 1)
                    degate(a1, st, st + 1)  # both -> gsems[16] >= 32
                h0 = h0p.tile([128, SZ], fp16, tag="h0")
                h1 = h1p.tile([22, SZ], fp16, tag="h1")
                nc.scalar.activation(h0[:], t0[:], AF.Relu, bias=b1p_sb[:, 0:1])
                nc.scalar.activation(h1[:], t1[:], AF.Relu, bias=b1p_sb[0:22, 1:2])
                ps2 = ps2p.tile([L, SZ], f32, tag="ps2")
                nc.tensor.matmul(
                    ps2[:], w2p_sb[:, 0:L], h0[:], start=True, stop=False
                )
                nc.tensor.matmul(
                    ps2[:], w2p_sb[0:22, L:2 * L], h1[:], start=False, stop=True
                )
                ot = otp.tile([L, SZ], f32)
                if st % 2 == 0:
                    nc.scalar.activation(ot[:], ps2[:], AF.Identity, bias=b2p_sb[:])
                else:
                    nc.vector.tensor_scalar_add(out=ot[:], in0=ps2[:], scalar1=b2p_sb[:])
                sync_dma(
                    out=outT_d[:, st * SZ:(st + 1) * SZ], in_=ot[:]
                )

    # Strip Tile's prep-mode DMASW pre-bump instructions (~1.2us each on
    # the Pool engine, serialized with the preps). After the consumer-wait
    # surgery above the DMASW lane sems have no real waiters: consumers are
    # gated by the per-gather descriptor sems, and the final Drain's DMASW
    # waits are subsumed by its engine-clock waits (every gather's data
    # flows through DVE->ACT->out-DMA, all upstream of the drain). Zero any
    # remaining DMASW waits so nothing blocks on the now-never-bumped sems.
    for b in nc.main_func.blocks:
        il = b.instructions
        for i in [x for x in il if type(x).__name__ == "InstIncSwdgeSem"]:
            il.remove(i)
        for i in il:
            si = i.sync_info
            if si is None:
                continue
            for w in (si.on_wait or []):
                if w.ant_name and w.ant_name.startswith("DMASW"):
                    w.wait_value = 0

    return nc


def _wrap_idx(idx_1d):
    """SWDGE index layout: [16, n/16].T wrapped, replicated to 128 rows."""
    n = len(idx_1d)
    arr16 = idx_1d.reshape(n // 16, 16).T
    return np.tile(arr16, (8, 1)).astype(np.int16)


def prep_inputs(xs, spans, batch_ids, W1, b1, W2, b2):
    """Host-side routing and layout. Returns (in_maps, slot_maps)."""
    xs = np.asarray(xs, dtype=np.float32)
    spans = np.asarray(spans).astype(np.int64)
    batch_ids = np.asarray(batch_ids).astype(np.int64)
    W1 = np.asarray(W1, dtype=np.float32)
    b1 = np.asarray(b1, dtype=np.float32)
    W2 = np.asarray(W2, dtype=np.float32)
    b2 = np.asarray(b2, dtype=np.float32)

    core = batch_ids // BPC
    local0 = (batch_ids % BPC) * T + spans[:, 0]
    local1 = (batch_ids % BPC) * T + spans[:, 1]

    # shared weights
    W1h = W1.astype(np.float16)
    wc = np.empty((128, N_KB * W1N), np.float16)
    for kb in range(N_KB):
        wc[:, kb * W1N:kb * W1N + H] = W1h[kb * 128:(kb + 1) * 128, :]
        wc[:, kb * W1N + H:(kb + 1) * W1N] = W1h[D + kb * 128:D + (kb + 1) * 128, :]
    w2p = np.zeros((128, 2 * L), np.float16)
    w2p[:, 0:L] = W2[0:128].astype(np.float16)
    w2p[0:H - 128, L:2 * L] = W2[128:H].astype(np.float16)
    b1p = np.zeros((128, 2), np.float32)
    b1p[:, 0] = b1[0:128]
    b1p[0:H - 128, 1] = b1[128:H]
    b2p = np.ascontiguousarray(b2.reshape(L, 1))
    iota = np.arange(128, dtype=np.uint8).reshape(128, 1)

    in_maps = []
    slot_maps = []
    for c in range(NCORES):
        sel = np.nonzero(core == c)[0]
        i0 = local0[sel]
        i1 = local1[sel]
        blk = i0 >> 7

        slot_map = np.full(MPAD, -1, np.int64)
        s_i0 = np.zeros(MPAD, np.int64)
        s_i1 = np.zeros(MPAD, np.int64)
        overflow = []
        for b in range(NBT):
            ids = np.nonzero(blk == b)[0]
            take = ids[:SZ]
            overflow.append(ids[SZ:])
            base = b * SZ
            ntk = len(take)
            slot_map[base:base + ntk] = sel[take]
            s_i0[base:base + ntk] = i0[take]
            s_i1[base:base + ntk] = i1[take]
            # pad: i0 at block start (one-hot row 0), i1 = 0
            s_i0[base + ntk:base + SZ] = b * 128
        ov = np.concatenate(overflow) if overflow else np.empty(0, np.int64)
        assert len(ov) <= SZ, f"core {c}: overflow {len(ov)} > {SZ}"
        base = NBT * SZ
        nov = len(ov)
        slot_map[base:base + nov] = sel[ov]
        s_i0[base:base + nov] = i0[ov]
        s_i1[base:base + nov] = i1[ov]

        # one-hot compare values: block-relative begin-token, u8, single row
        i0v = np.ascontiguousarray(
            (s_i0[:NBT * SZ] & 127).astype(np.uint8).reshape(1, NBT * SZ)
        )
        idxg = _wrap_idx(s_i1)
        idxa = _wrap_idx(s_i0[NBT * SZ:])

        # xsT: [p, ch, kb, t] = xs[token=ch*256+t, d=kb*128+p], fp16
        xs_c = xs[c * BPC:(c + 1) * BPC].reshape(TC, D).astype(np.float16)
        xsT = np.ascontiguousarray(
            xs_c.T.reshape(N_KB, 128, N_CH, 256).transpose(1, 2, 0, 3)
        )

        in_maps.append({
            "xsT": xsT, "wc": wc, "w2p": w2p,
            "b1p": b1p, "b2p": b2p, "iota": iota, "i0v": i0v,
            "idxg": idxg, "idxa": idxa,
        })
        slot_maps.append(slot_map)

    return in_maps, slot_maps


def _scatter_out(results, slot_maps):
    out = np.empty((N_SPANS, L), np.float32)
    for c in range(NCORES):
        sm = slot_maps[c]
        valid = sm >= 0
        out[sm[valid]] = results[c]["outT"].T[valid]
    return out


def _install_ntff_shim():
    """Provide antenv.axon_hooks (missing on this image) so that
    run_bass_kernel_spmd(trace=True) can drive NTFF profiling via the
    axon .so. Only used by the profiling path."""
    import sys
    import types
    import ctypes
    import contextlib

    if "antenv.axon_hooks" in sys.modules:
        return
    import antenv

    holder = {"hook": None}
    mod = types.ModuleType("antenv.axon_hooks")
    mod.set_axon_ntff_profile_hook = lambda h: holder.__setitem__("hook", h)
    mod.get_axon_ntff_profile_hook = lambda: holder["hook"]
    sys.modules["antenv.axon_hooks"] = mod
    antenv.axon_hooks = mod

    so_path = "/opt/axon/libaxon_pjrt.so"
    try:
        lib = ctypes.CDLL(so_path)
    except OSError:
        return
    if not hasattr(lib, "axon_start_nrt_profile"):
        return
    lib.axon_start_nrt_profile.argtypes = [
        ctypes.POINTER(ctypes.c_int64),
        ctypes.c_size_t,
    ]
    lib.axon_start_nrt_profile.restype = ctypes.c_int64
    lib.axon_stop_nrt_profile.argtypes = [ctypes.c_char_p]
    lib.axon_stop_nrt_profile.restype = ctypes.c_int64

    @contextlib.contextmanager
    def _hook(output_dir, device_ids):
        import jax

        jax.devices()
        if device_ids:
            ids = (ctypes.c_int64 * len(device_ids))(*device_ids)
            rc = lib.axon_start_nrt_profile(ids, len(device_ids))
        else:
            rc = lib.axon_start_nrt_profile(None, 0)
        if rc != 0:
            raise RuntimeError(f"axon_start_nrt_profile rc={rc}")
        try:
            yield
        finally:
            n = lib.axon_stop_nrt_profile(str(output_dir).encode())
            print(f"profile: {n} file(s) written to {output_dir}")

    mod.set_axon_ntff_profile_hook(_hook)


def run(inputs: dict, trace: bool = False):
    """Run on the 8 NeuronCores. Returns (out, BassKernelResults)."""
    from concourse import bass_utils
    from concourse.bass_utils import run_bass_kernel_spmd

    if trace:
        _install_ntff_shim()
        bass_utils.upload_artifacts = lambda tmpdir: str(tmpdir)

    in_maps, slot_maps = prep_inputs(**inputs)
    nc = build_graph()
    nc.finalize()
    res = run_bass_kernel_spmd(
        nc, in_maps, list(range(NCORES)), trace=trace
    )
    return _scatter_out(res.results, slot_maps), res


def kernel(**inputs) -> np.ndarray:
    out, _ = run(inputs, trace=False)
    return out


# revision 23
# speedup vs baseline: 1.2808x; 1.1405x over previous
"""Trainium2 Bass kernel for nn_BaselineSpanScorer (span-pair MLP scorer), v6.

reference:
    xs        [32, 512, 1024] f32
    spans     [65536, 2] int   (begin/end token index within sequence)
    batch_ids [65536] int
    W1 [2048, 150], b1 [150], W2 [150, 17], b2 [17]
    out[n] = relu(concat(xs[b, s0], xs[b, s1]) @ W1 + b1) @ W2 + b2

Strategy (8 NeuronCores, data parallel, per-core graph is input-shape fixed):
  - Shard xs by batch: core c owns batches [4c, 4c+4) = 2048 token rows.
  - Factorize: A[t] = xs[t] @ W1[:1024], G[t] = xs[t] @ W1[1024:], so
    pre[n] = A[i0_n] + G[i1_n] + b1 (token reuse cuts matmul work ~4x).
  - Stage 1: 16 token tiles x 8 K-blocks of fused [A|G] 300-col fp16
    matmuls. A rows stay in SBUF (one-hot lhsT + SBUF-source overflow
    gather); G rows go to a 1MB DRAM table (512B rows).
  - Spans are bucketed by i0 token block (16 buckets of 512 + 1 overflow
    tile, MPAD=8704): a block tile's one-hot contraction needs exactly
    ONE 128-token window. Overflow spans gather A from SBUF.
  - G side via 10 DRAM-source transpose dma_gathers of 896 idxs each
    (s2m ring descriptors 114 <= 128, the real per-op limit; fewer, fatter
    gathers cut the ~2us/op Pool-engine descriptor-gen serial cost).
    All 11 gathers are PREPARED during the xs load and fired by per-gather
    trigger_dma(count=1) once the table lands. Tile's prep-mode DMASW
    plumbing is bypassed: pre-bump insts stripped post-build, consumers
    wait on per-gather descriptor-completion sems instead.
  - Stage 2 per 512-span tile: PSUM accumulates onehot-A (or SBUF-gathered
    A); gathered G added on DVE (split in two where a 512-tile crosses an
    896-gather-chunk boundary); relu+bias on ACT; [150]x[17] W2
    contraction; bias; DMA out scores^T.
  - Host scatters per-core outputs back to original span order.

Compute dtype fp16 (rel err ~1e-3 vs f32 reference), f32 output.
"""

import os

os.environ.setdefault("MYCRO_LOCAL_CACHE", "1")

import numpy as np

# ---------------- problem constants (hardcoded per spec) ----------------
B, T, D = 32, 512, 1024
N_SPANS = 65536
H, L = 150, 17
NCORES = 8
BPC = B // NCORES        # batches per core = 4
TC = BPC * T             # tokens per core = 2048
N_KB = D // 128          # K blocks in stage 1 = 8
N_TT = TC // 128         # token tiles in stage 1 = 16
N_CH = 8                 # xsT load chunks (2 token tiles each)
SZ = 512                 # spans per stage-2 tile
NBT = TC // 128          # block tiles = 16
NT = NBT + 1             # + 1 overflow tile
MPAD = NT * SZ           # padded span count per core = 8704
HP2 = 256                # table row elems (fp16 -> 512B)
W1N = 2 * H              # 300: stage-1 moving operand width (A | G)
GSZ = 896                # G-gather idx count (s2m = 896/16*2+2 = 114 <= 128)
NG = -(-MPAD // GSZ)     # G-gather chunks = 10
GPAD = NG * GSZ          # padded G-idx stream = 8960


def build_graph():
    """Build the per-core SPMD Bass graph (fixed shapes, input independent)."""
    from concourse import bacc
    import concourse.mybir as mybir
    from concourse.tile import TileContext
    from concourse import library_config
    from concourse.tile_rust import add_dep_helper

    fp16 = mybir.dt.float16
    f32 = mybir.dt.float32
    i16 = mybir.dt.int16
    u8 = mybir.dt.uint8
    AF = mybir.ActivationFunctionType
    EQ = mybir.AluOpType.is_equal
    ADD = mybir.AluOpType.add

    nc = bacc.Bacc(num_swdge_queues=4)

    _chain = []

    def sync_dma(**kw):
        g = nc.sync.dma_start(**kw)
        if _chain:
            add_dep_helper(g.ins, _chain[-1].ins, False, "pin dma order")
        _chain.append(g)
        return g

    xsT_d = nc.declare_dram_parameter("xsT", [128, N_CH, N_KB, 256], fp16, isOutput=False)
    wc_d = nc.declare_dram_parameter("wc", [128, N_KB * W1N], fp16, isOutput=False)
    w2p_d = nc.declare_dram_parameter("w2p", [128, 2 * L], fp16, isOutput=False)
    b1p_d = nc.declare_dram_parameter("b1p", [128, 2], f32, isOutput=False)
    b2p_d = nc.declare_dram_parameter("b2p", [L, 1], f32, isOutput=False)
    iota_d = nc.declare_dram_parameter("iota", [128, 1], u8, isOutput=False)
    i0v_d = nc.declare_dram_parameter("i0v", [128, NBT * SZ], u8, isOutput=False)
    idxg_d = nc.declare_dram_parameter("idxg", [128, NG * GSZ // 16], i16, isOutput=False)
    idxa_d = nc.declare_dram_parameter("idxa", [128, 32], i16, isOutput=False)
    outT_d = nc.declare_dram_parameter("outT", [L, MPAD], f32, isOutput=True)

    with TileContext(nc) as tc:
        with (
            tc.tile_pool(name="const", bufs=1) as constp,
            tc.tile_pool(name="xst", bufs=1) as xstp,
            tc.tile_pool(name="asb", bufs=1) as asbp,
            tc.tile_pool(name="gt", bufs=2) as gtp,
            tc.tile_pool(name="dram", bufs=1, space="DRAM") as dramp,
            tc.tile_pool(name="s0", bufs=1) as s0p,
            tc.tile_pool(name="gg", bufs=1) as ggp,
            tc.tile_pool(name="psX", bufs=5, space="PSUM") as psXp,
            tc.tile_pool(name="ps2", bufs=3, space="PSUM") as ps2p,
            tc.tile_pool(name="h0", bufs=6) as h0p,
            tc.tile_pool(name="t0", bufs=6) as t0p,
            tc.tile_pool(name="t1", bufs=6) as t1p,
            tc.tile_pool(name="h1", bufs=6) as h1p,
            tc.tile_pool(name="ot", bufs=8) as otp,
        ):
            nc.gpsimd.load_library(library_config.mlp)

            # ---- input loads ----
            # stage-1 weights first (PE needs them at t~12us), then the
            # small gather-index tensors (Pool-engine desc-gen needs them
            # early), then the bulk xs^T chunks + one-hot compare rows.
            wc_lo = constp.tile([128, 4 * W1N], fp16)
            wc_hi = constp.tile([128, 4 * W1N], fp16)
            wc_dma = sync_dma(out=wc_lo[:], in_=wc_d[:, 0:4 * W1N])
            idxg_sb = constp.tile([128, NG * GSZ // 16], i16)
            sync_dma(out=idxg_sb[:], in_=idxg_d[:])
            idxa_sb = constp.tile([128, 32], i16)
            sync_dma(out=idxa_sb[:], in_=idxa_d[:])
            iota_sb = constp.tile([128, 1], u8)
            sync_dma(out=iota_sb[:], in_=iota_d[:])
            sync_dma(out=wc_hi[:], in_=wc_d[:, 4 * W1N:])
            xst_sb = xstp.tile([128, N_CH, N_KB, 256], fp16)
            i0v_sb = constp.tile([128, NBT * SZ], u8)
            for ch in range(N_CH):
                sync_dma(out=xst_sb[:, ch, :, :], in_=xsT_d[:, ch, :, :])
                if ch == 3:
                    # one-hot compare rows mid-stream: needed by DVE IS_EQ,
                    # which runs during the load
                    sync_dma(out=i0v_sb[:], in_=i0v_d[:])
            w2p_sb = constp.tile([128, 2 * L], fp16)
            sync_dma(out=w2p_sb[:], in_=w2p_d[:])
            b1p_sb = constp.tile([128, 2], f32)
            sync_dma(out=b1p_sb[:], in_=b1p_d[:])
            b2p_sb = constp.tile([L, 1], f32)
            sync_dma(out=b2p_sb[:], in_=b2p_d[:])

            # ---- A rows: SBUF-resident table (one-hot lhsT + SBUF-source
            # overflow gather). token tt*128+p -> partition p, rank tt. ----
            a_sb = asbp.tile([128, N_TT, HP2], fp16)
            # ---- G rows: DRAM table, 512B rows ----
            tab = dramp.tile([TC, HP2], fp16)

            # ---- A-side one-hot tiles (DVE; overlapped with stage 1) ----
            s0_tiles = []
            for b in range(NBT):
                s0 = s0p.tile([128, SZ], fp16, tag=f"s0_{b}", name=f"s0_{b}")
                nc.vector.tensor_tensor(
                    out=s0[:],
                    in0=iota_sb[:, 0:1].to_broadcast([128, SZ]),
                    in1=i0v_sb[:, b * SZ:(b + 1) * SZ],
                    op=EQ,
                )
                s0_tiles.append(s0)

            # ---- stage 1: xs @ [W1a | W1g] -> a_sb (SBUF) + tab (DRAM) ----
            tab_writers = []
            last_tab_dma = None
            for tt in range(N_TT):
                ch, hf = tt // 2, tt % 2
                ps = psXp.tile([128, 512], f32, tag="ps")
                for kb in range(N_KB):
                    wch = wc_lo if kb < 4 else wc_hi
                    nc.tensor.matmul(
                        ps[:, 0:W1N],
                        xst_sb[:, ch, kb, hf * 128:(hf + 1) * 128],
                        wch[:, (kb % 4) * W1N:(kb % 4 + 1) * W1N],
                        start=(kb == 0),
                        stop=(kb == N_KB - 1),
                    )
                tab_writers.append(
                    nc.vector.memset(a_sb[:, tt, H:HP2], 0.0)
                )
                tab_writers.append(
                    nc.scalar.activation(a_sb[:, tt, 0:H], ps[:, 0:H], AF.Copy)
                )
                gt = gtp.tile([128, HP2], fp16, tag="gt")
                nc.vector.memset(gt[:, H:HP2], 0.0)
                nc.scalar.activation(gt[:, 0:H], ps[:, H:W1N], AF.Copy)
                # same-queue FIFO: the last table write covers all 16
                last_tab_dma = sync_dma(
                    out=tab[tt * 128:(tt + 1) * 128, :], in_=gt[:]
                )
                tab_writers.append(last_tab_dma)
            tw_by_name = {w.ins.name: w.ins for w in tab_writers}

            # ---- gather preps: descriptor-gen on Pool, overlapped with the
            # xs load. Emitted AFTER the table writes so the RAW edges are
            # emission-time visible, then demoted to no-sync on the prep
            # (desc-gen doesn't read the table) and re-attached as sync deps
            # of the triggers. ----
            gg_sb = ggp.tile([128, NG, 2, GSZ], fp16)
            ggA = ggp.tile([128, 2, SZ], fp16, tag="ggA", name="ggA")
            gsems = [nc.alloc_semaphore(f"ggs{k}") for k in range(NG + 1)]
            preps = []
            prev_g = None

            def post_prep(g):
                nonlocal prev_g
                for dep in set(g.ins.sync_dependency_names()):
                    if dep in tw_by_name:
                        g.ins.try_remove_dependency(dep)
                        add_dep_helper(g.ins, tw_by_name[dep], False, "defer src")
                if prev_g is not None:
                    add_dep_helper(g.ins, prev_g.ins, False, "pin prep order")
                prev_g = g
                preps.append(g)

            for k in range(NG):
                post_prep(nc.gpsimd.dma_gather(
                    gg_sb[:, k, :, :],
                    tab[:],
                    idxg_sb[:, k * (GSZ // 16):(k + 1) * (GSZ // 16)],
                    GSZ,
                    GSZ,
                    elem_size=HP2,
                    elem_step=HP2,
                    transpose=True,
                    queue_num=k % 4,
                    prepare_only=True,
                    sem=gsems[k],
                ))
            # overflow A rows from the SBUF a-table
            post_prep(nc.gpsimd.dma_gather(
                ggA[:],
                a_sb[:],
                idxa_sb[:],
                SZ,
                SZ,
                elem_size=HP2,
                transpose=True,
                queue_num=NG % 4,
                prepare_only=True,
                sem=gsems[NG],
                sbuf_tokens_per_rank=128,
                sbuf_free_dim_per_rank=2 * HP2,
                sbuf_byte_offset=0,
            ))
            prep_by_name = {p.ins.name: p.ins for p in preps}

            # ---- fire the prepared gathers (table complete) ----
            # One count=1 trigger per gather, in order. Only the first
            # needs the table sync deps -- Pool executes the rest after it.
            for k, p in enumerate(preps):
                q = k % 4 if k < NG else NG % 4
                trig = nc.gpsimd.trigger_dma(count=1, queue_num=q)
                add_dep_helper(trig.ins, p.ins, False, "after prep")
                if k == 0:
                    add_dep_helper(
                        trig.ins, last_tab_dma.ins, True, "tab -> trigger"
                    )
                    for w in tab_writers:
                        if w.ins.name != last_tab_dma.ins.name:
                            add_dep_helper(trig.ins, w.ins, True, "a_sb -> trig")

            def degate(inst, *waits):
                """Swap Tile's prep-write attribution for explicit waits on
                the per-gather DMA-completion sems."""
                for dep in set(inst.ins.sync_dependency_names()):
                    if dep in prep_by_name:
                        inst.ins.try_remove_dependency(dep)
                        add_dep_helper(
                            inst.ins, prep_by_name[dep], False, "manual sem"
                        )
                for s in waits:
                    inst.wait_op(gsems[s], 16, "sem-ge")

            # G data for global span s lives at chunk s//GSZ, col s%GSZ.
            def g_ap(st, lo_hi, c0, c1):
                s = st * SZ + c0
                k, off = s // GSZ, s % GSZ
                assert off + (c1 - c0) <= GSZ
                if lo_hi == 0:
                    return k, gg_sb[:, k, 0, off:off + (c1 - c0)]
                return k, gg_sb[0:22, k, 1, off:off + (c1 - c0)]

            # ---- stage 2 ----
            for st in range(NT):
                # split points where this 512-tile crosses an 896-chunk edge
                lo, hi = st * SZ, (st + 1) * SZ
                cut = -(-lo // GSZ) * GSZ      # next chunk edge >= lo
                segs = [(0, cut - lo), (cut - lo, SZ)] if lo < cut < hi \
                    else [(0, SZ)]
                t0 = t0p.tile([128, SZ], fp16, tag="t0")
                t1 = t1p.tile([22, SZ], fp16, tag="t1")
                if st < NBT:
                    pA0 = psXp.tile([128, SZ], f32, tag="ps")
                    nc.tensor.matmul(
                        pA0[:], a_sb[:, st, 0:128], s0_tiles[st][:],
                        start=True, stop=True,
                    )
                    pA1 = psXp.tile([22, SZ], f32, tag="ps")
                    nc.tensor.matmul(
                        pA1[:], a_sb[:, st, 128:H], s0_tiles[st][:],
                        start=True, stop=True,
                    )
                    for c0, c1 in segs:
                        k, gap = g_ap(st, 0, c0, c1)
                        a0 = nc.vector.tensor_tensor(
                            out=t0[:, c0:c1], in0=pA0[:, c0:c1], in1=gap,
                            op=ADD,
                        )
                        degate(a0, k)
                        k, gap = g_ap(st, 1, c0, c1)
                        a1 = nc.vector.tensor_tensor(
                            out=t1[:, c0:c1], in0=pA1[:, c0:c1], in1=gap,
                            op=ADD,
                        )
                        degate(a1, k)
                else:
                    # overflow tile: A side gathered from SBUF (ggA). The
                    # ggA wait rides a tiny same-engine memset (sync-dep
                    # chained) to stay within the per-inst wait-slot limit.
                    xw = nc.vector.memset(t0[0:1, 0:1], 0.0)
                    xw.wait_op(gsems[NG], 16, "sem-ge")
                    for dep in set(xw.ins.sync_dependency_names()):
                        if dep in prep_by_name:
                            xw.ins.try_remove_dependency(dep)
                    for c0, c1 in segs:
                        k, gap = g_ap(st, 0, c0, c1)
                        a0 = nc.vector.tensor_tensor(
                            out=t0[:, c0:c1], in0=ggA[:, 0, c0:c1], in1=gap,
                            op=ADD,
                        )
                        degate(a0, k)
                        add_dep_helper(a0.ins, xw.ins, True, "ggA wait")
                        k, gap = g_ap(st, 1, c0, c1)
                        a1 = nc.vector.tensor_tensor(
                            out=t1[:, c0:c1], in0=ggA[0:22, 1, c0:c1], in1=gap,
                            op=ADD,
                        )
                        degate(a1, k)
                        add_dep_helper(a1.ins, xw.ins, True, "ggA wait")
                h0 = h0p.tile([128, SZ], fp16, tag="h0")
                h1 = h1p.tile([22, SZ], fp16, tag="h1")
                nc.scalar.activation(h0[:], t0[:], AF.Relu, bias=b1p_sb[:, 0:1])
                nc.scalar.activation(h1[:], t1[:], AF.Relu, bias=b1p_sb[0:22, 1:2])
                ps2 = ps2p.tile([L, SZ], f32, tag="ps2")
                nc.tensor.matmul(
                    ps2[:], w2p_sb[:, 0:L], h0[:], start=True, stop=False
                )
                nc.tensor.matmul(
                    ps2[:], w2p_sb[0:22, L:2 * L], h1[:], start=False, stop=True
                )
                ot = otp.tile([L, SZ], f32)
                if st % 2 == 0:
                    nc.scalar.activation(ot[:], ps2[:], AF.Identity, bias=b2p_sb[:])
                else:
                    nc.vector.tensor_scalar_add(out=ot[:], in0=ps2[:], scalar1=b2p_sb[:])
                sync_dma(
                    out=outT_d[:, st * SZ:(st + 1) * SZ], in_=ot[:]
                )

    # Strip Tile's prep-mode DMASW pre-bump instructions (~1.2us each on
    # the Pool engine, serialized with the preps). After the consumer-wait
    # surgery above the DMASW lane sems have no real waiters: consumers are
    # gated by the per-gather descriptor sems, and the final Drain's DMASW
    # waits are subsumed by its engine-clock waits (every gather's data
    # flows through DVE->ACT->out-DMA, all upstream of the drain). Zero any
    # remaining DMASW waits so nothing blocks on the now-never-bumped sems.
    for b in nc.main_func.blocks:
        il = b.instructions
        for i in [x for x in il if type(x).__name__ == "InstIncSwdgeSem"]:
            il.remove(i)
        for i in il:
            si = i.sync_info
            if si is None:
                continue
            for w in (si.on_wait or []):
                if w.ant_name and w.ant_name.startswith("DMASW"):
                    w.wait_value = 0

    return nc


def _wrap_idx(idx_1d):
    """SWDGE index layout: [16, n/16].T wrapped, replicated to 128 rows."""
    n = len(idx_1d)
    arr16 = idx_1d.reshape(n // 16, 16).T
    return np.tile(arr16, (8, 1)).astype(np.int16)


def prep_inputs(xs, spans, batch_ids, W1, b1, W2, b2):
    """Host-side routing and layout. Returns (in_maps, slot_maps)."""
    xs = np.asarray(xs, dtype=np.float32)
    spans = np.asarray(spans).astype(np.int64)
    batch_ids = np.asarray(batch_ids).astype(np.int64)
    W1 = np.asarray(W1, dtype=np.float32)
    b1 = np.asarray(b1, dtype=np.float32)
    W2 = np.asarray(W2, dtype=np.float32)
    b2 = np.asarray(b2, dtype=np.float32)

    core = batch_ids // BPC
    local0 = (batch_ids % BPC) * T + spans[:, 0]
    local1 = (batch_ids % BPC) * T + spans[:, 1]

    # shared weights
    W1h = W1.astype(np.float16)
    wc = np.empty((128, N_KB * W1N), np.float16)
    for kb in range(N_KB):
        wc[:, kb * W1N:kb * W1N + H] = W1h[kb * 128:(kb + 1) * 128, :]
        wc[:, kb * W1N + H:(kb + 1) * W1N] = W1h[D + kb * 128:D + (kb + 1) * 128, :]
    w2p = np.zeros((128, 2 * L), np.float16)
    w2p[:, 0:L] = W2[0:128].astype(np.float16)
    w2p[0:H - 128, L:2 * L] = W2[128:H].astype(np.float16)
    b1p = np.zeros((128, 2), np.float32)
    b1p[:, 0] = b1[0:128]
    b1p[0:H - 128, 1] = b1[128:H]
    b2p = np.ascontiguousarray(b2.reshape(L, 1))
    iota = np.arange(128, dtype=np.uint8).reshape(128, 1)

    in_maps = []
    slot_maps = []
    for c in range(NCORES):
        sel = np.nonzero(core == c)[0]
        i0 = local0[sel]
        i1 = local1[sel]
        blk = i0 >> 7

        slot_map = np.full(MPAD, -1, np.int64)
        s_i0 = np.zeros(MPAD, np.int64)
        s_i1 = np.zeros(MPAD, np.int64)
        overflow = []
        for b in range(NBT):
            ids = np.nonzero(blk == b)[0]
            take = ids[:SZ]
            overflow.append(ids[SZ:])
            base = b * SZ
            ntk = len(take)
            slot_map[base:base + ntk] = sel[take]
            s_i0[base:base + ntk] = i0[take]
            s_i1[base:base + ntk] = i1[take]
            # pad: i0 at block start (one-hot row 0), i1 = 0
            s_i0[base + ntk:base + SZ] = b * 128
        ov = np.concatenate(overflow) if overflow else np.empty(0, np.int64)
        assert len(ov) <= SZ, f"core {c}: overflow {len(ov)} > {SZ}"
        base = NBT * SZ
        nov = len(ov)
        slot_map[base:base + nov] = sel[ov]
        s_i0[base:base + nov] = i0[ov]
        s_i1[base:base + nov] = i1[ov]

        # one-hot compare values: block-relative begin-token, u8
        i0rel = (s_i0[:NBT * SZ] & 127).astype(np.uint8)
        i0v = np.ascontiguousarray(np.broadcast_to(i0rel, (128, NBT * SZ)))
        # G-gather idx stream: MPAD values padded to NG*GSZ, wrapped per chunk
        gidx = np.zeros(GPAD, np.int64)
        gidx[:MPAD] = s_i1
        idxg = np.concatenate(
            [_wrap_idx(gidx[k * GSZ:(k + 1) * GSZ]) for k in range(NG)], axis=1
        )
        idxa = _wrap_idx(s_i0[NBT * SZ:])

        # xsT: [p, ch, kb, t] = xs[token=ch*256+t, d=kb*128+p], fp16
        xs_c = xs[c * BPC:(c + 1) * BPC].reshape(TC, D).astype(np.float16)
        xsT = np.ascontiguousarray(
            xs_c.T.reshape(N_KB, 128, N_CH, 256).transpose(1, 2, 0, 3)
        )

        in_maps.append({
            "xsT": xsT, "wc": wc, "w2p": w2p,
            "b1p": b1p, "b2p": b2p, "iota": iota, "i0v": i0v,
            "idxg": idxg, "idxa": idxa,
        })
        slot_maps.append(slot_map)

    return in_maps, slot_maps


def _scatter_out(results, slot_maps):
    out = np.empty((N_SPANS, L), np.float32)
    for c in range(NCORES):
        sm = slot_maps[c]
        valid = sm >= 0
        out[sm[valid]] = results[c]["outT"].T[valid]
    return out


def _install_ntff_shim():
    """Provide antenv.axon_hooks (missing on this image) so that
    run_bass_kernel_spmd(trace=True) can drive NTFF profiling via the
    axon .so. Only used by the profiling path."""
    import sys
    import types
    import ctypes
    import contextlib

    if "antenv.axon_hooks" in sys.modules:
        return
    import antenv

    holder = {"hook": None}
    mod = types.ModuleType("antenv.axon_hooks")
    mod.set_axon_ntff_profile_hook = lambda h: holder.__setitem__("hook", h)
    mod.get_axon_ntff_profile_hook = lambda: holder["hook"]
    sys.modules["antenv.axon_hooks"] = mod
    antenv.axon_hooks = mod

    so_path = "/opt/axon/libaxon_pjrt.so"
    try:
        lib = ctypes.CDLL(so_path)
    except OSError:
        return
    if not hasattr(lib, "axon_start_nrt_profile"):
        return
    lib.axon_start_nrt_profile.argtypes = [
        ctypes.POINTER(ctypes.c_int64),
        ctypes.c_size_t,
    ]
    lib.axon_start_nrt_profile.restype = ctypes.c_int64
    lib.axon_stop_nrt_profile.argtypes = [ctypes.c_char_p]
    lib.axon_stop_nrt_profile.restype = ctypes.c_int64

    @contextlib.contextmanager
    def _hook(output_dir, device_ids):
        import jax

        jax.devices()
        if device_ids:
            ids = (ctypes.c_int64 * len(device_ids))(*device_ids)
            rc = lib.axon_start_nrt_profile(ids, len(device_ids))
        else:
            rc = lib.axon_start_nrt_profile(None, 0)
        if rc != 0:
            raise RuntimeError(f"axon_start_nrt_profile rc={rc}")
        try:
            yield
        finally:
            n = lib.axon_stop_nrt_profile(str(output_dir).encode())
            print(f"profile: {n} file(s) written to {output_dir}")

    mod.set_axon_ntff_profile_hook(_hook)


def run(inputs: dict, trace: bool = False):
    """Run on the 8 NeuronCores. Returns (out, BassKernelResults)."""
    from concourse import bass_utils
    from concourse.bass_utils import run_bass_kernel_spmd

    if trace:
        _install_ntff_shim()
        bass_utils.upload_artifacts = lambda tmpdir: str(tmpdir)

    in_maps, slot_maps = prep_inputs(**inputs)
    nc = build_graph()
    nc.finalize()
    res = run_bass_kernel_spmd(
        nc, in_maps, list(range(NCORES)), trace=trace
    )
    return _scatter_out(res.results, slot_maps), res


def kernel(**inputs) -> np.ndarray:
    out, _ = run(inputs, trace=False)
    return out


# revision 25
# speedup vs baseline: 1.4143x; 1.1042x over previous
"""Trainium2 Bass kernel for nn_BaselineSpanScorer (span-pair MLP scorer), v6.

reference:
    xs        [32, 512, 1024] f32
    spans     [65536, 2] int   (begin/end token index within sequence)
    batch_ids [65536] int
    W1 [2048, 150], b1 [150], W2 [150, 17], b2 [17]
    out[n] = relu(concat(xs[b, s0], xs[b, s1]) @ W1 + b1) @ W2 + b2

Strategy (8 NeuronCores, data parallel, per-core graph is input-shape fixed):
  - Shard xs by batch: core c owns batches [4c, 4c+4) = 2048 token rows.
  - Factorize: A[t] = xs[t] @ W1[:1024], G[t] = xs[t] @ W1[1024:], so
    pre[n] = A[i0_n] + G[i1_n] + b1 (token reuse cuts matmul work ~4x).
  - Stage 1: 16 token tiles x 8 K-blocks of fused [A|G] 300-col fp16
    matmuls. A rows stay in SBUF (one-hot lhsT + SBUF-source overflow
    gather); G rows go to a 1MB DRAM table (512B rows).
  - Spans are bucketed by i0 token block (16 buckets of 512 + 1 overflow
    tile, MPAD=8704): a block tile's one-hot contraction needs exactly
    ONE 128-token window. Overflow spans gather A from SBUF.
  - G side via 10 DRAM-source transpose dma_gathers of 896 idxs each
    (s2m ring descriptors 114 <= 128, the real per-op limit; fewer, fatter
    gathers cut the ~2us/op Pool-engine descriptor-gen serial cost).
    All 11 gathers are PREPARED during the xs load and fired by per-gather
    trigger_dma(count=1) once the table lands. Tile's prep-mode DMASW
    plumbing is bypassed: pre-bump insts stripped post-build, consumers
    wait on per-gather descriptor-completion sems instead.
  - Stage 2 per 512-span tile: PSUM accumulates onehot-A (or SBUF-gathered
    A); gathered G added on DVE (split in two where a 512-tile crosses an
    896-gather-chunk boundary); relu+bias on ACT; [150]x[17] W2
    contraction; bias; DMA out scores^T.
  - Host scatters per-core outputs back to original span order.

Compute dtype fp16 (rel err ~1e-3 vs f32 reference), f32 output.
"""

import os

os.environ.setdefault("MYCRO_LOCAL_CACHE", "1")

import numpy as np

# ---------------- problem constants (hardcoded per spec) ----------------
B, T, D = 32, 512, 1024
N_SPANS = 65536
H, L = 150, 17
NCORES = 8
BPC = B // NCORES        # batches per core = 4
TC = BPC * T             # tokens per core = 2048
N_KB = D // 128          # K blocks in stage 1 = 8
N_TT = TC // 128         # token tiles in stage 1 = 16
N_CH = 8                 # xsT load chunks (2 token tiles each)
SZ = 512                 # spans per stage-2 tile
NBT = TC // 128          # block tiles = 16
NT = NBT + 1             # + 1 overflow tile
MPAD = NT * SZ           # padded span count per core = 8704
HP2 = 256                # table row elems (fp16 -> 512B)
W1N = 2 * H              # 300: stage-1 moving operand width (A | G)
GSZ = 896                # G-gather idx count (s2m = 896/16*2+2 = 114 <= 128)
NG = -(-MPAD // GSZ)     # G-gather chunks = 10
GPAD = NG * GSZ          # padded G-idx stream = 8960


def build_graph():
    """Build the per-core SPMD Bass graph (fixed shapes, input independent)."""
    from concourse import bacc
    import concourse.mybir as mybir
    from concourse.tile import TileContext
    from concourse import library_config
    from concourse.tile_rust import add_dep_helper

    fp16 = mybir.dt.float16
    f32 = mybir.dt.float32
    i16 = mybir.dt.int16
    u8 = mybir.dt.uint8
    AF = mybir.ActivationFunctionType
    EQ = mybir.AluOpType.is_equal
    ADD = mybir.AluOpType.add

    nc = bacc.Bacc(num_swdge_queues=4)

    _chain = []
    _schain = []

    def sync_dma(**kw):
        g = nc.sync.dma_start(**kw)
        if _chain:
            add_dep_helper(g.ins, _chain[-1].ins, False, "pin dma order")
        _chain.append(g)
        return g

    def scalar_dma(**kw):
        g = nc.scalar.dma_start(**kw)
        if _schain:
            add_dep_helper(g.ins, _schain[-1].ins, False, "pin dma order")
        _schain.append(g)
        return g

    xsT_d = nc.declare_dram_parameter("xsT", [128, N_CH, N_KB, 256], fp16, isOutput=False)
    wc_d = nc.declare_dram_parameter("wc", [128, N_KB * W1N], fp16, isOutput=False)
    w2p_d = nc.declare_dram_parameter("w2p", [128, 2 * L], fp16, isOutput=False)
    b1p_d = nc.declare_dram_parameter("b1p", [128, 2], f32, isOutput=False)
    b2p_d = nc.declare_dram_parameter("b2p", [L, 1], f32, isOutput=False)
    iota_d = nc.declare_dram_parameter("iota", [128, 1], u8, isOutput=False)
    i0v_d = nc.declare_dram_parameter("i0v", [128, NBT * SZ], u8, isOutput=False)
    idxg_d = nc.declare_dram_parameter("idxg", [128, NG * GSZ // 16], i16, isOutput=False)
    idxa_d = nc.declare_dram_parameter("idxa", [128, 32], i16, isOutput=False)
    outT_d = nc.declare_dram_parameter("outT", [L, MPAD], f32, isOutput=True)

    with TileContext(nc) as tc:
        with (
            tc.tile_pool(name="const", bufs=1) as constp,
            tc.tile_pool(name="xst", bufs=1) as xstp,
            tc.tile_pool(name="asb", bufs=1) as asbp,
            tc.tile_pool(name="gt", bufs=16) as gtp,
            tc.tile_pool(name="dram", bufs=1, space="DRAM") as dramp,
            tc.tile_pool(name="s0", bufs=1) as s0p,
            tc.tile_pool(name="gg", bufs=1) as ggp,
            tc.tile_pool(name="psX", bufs=5, space="PSUM") as psXp,
            tc.tile_pool(name="ps2", bufs=3, space="PSUM") as ps2p,
            tc.tile_pool(name="h0", bufs=6) as h0p,
            tc.tile_pool(name="t0", bufs=6) as t0p,
            tc.tile_pool(name="t1", bufs=6) as t1p,
            tc.tile_pool(name="h1", bufs=6) as h1p,
            tc.tile_pool(name="ot", bufs=8) as otp,
        ):
            nc.gpsimd.load_library(library_config.mlp)

            # ---- input loads ----
            # stage-1 weights first (PE needs them at t~12us), then the
            # small gather-index tensors (Pool-engine desc-gen needs them
            # early), then the bulk xs^T chunks + one-hot compare rows.
            wc_lo = constp.tile([128, 4 * W1N], fp16)
            wc_hi = constp.tile([128, 4 * W1N], fp16)
            wc_dma = sync_dma(out=wc_lo[:], in_=wc_d[:, 0:4 * W1N])
            idxg_sb = constp.tile([128, NG * GSZ // 16], i16)
            sync_dma(out=idxg_sb[:], in_=idxg_d[:])
            idxa_sb = constp.tile([128, 32], i16)
            sync_dma(out=idxa_sb[:], in_=idxa_d[:])
            iota_sb = constp.tile([128, 1], u8)
            sync_dma(out=iota_sb[:], in_=iota_d[:])
            sync_dma(out=wc_hi[:], in_=wc_d[:, 4 * W1N:])
            xst_sb = xstp.tile([128, N_CH, N_KB, 256], fp16)
            i0v_sb = constp.tile([128, NBT * SZ], u8)
            for ch in range(N_CH):
                sync_dma(out=xst_sb[:, ch, :, :], in_=xsT_d[:, ch, :, :])
            # one-hot compare rows after the bulk load: DVE IS_EQ output is
            # first consumed by stage-2 matmuls, well after the load
            sync_dma(out=i0v_sb[:], in_=i0v_d[:])
            w2p_sb = constp.tile([128, 2 * L], fp16)
            sync_dma(out=w2p_sb[:], in_=w2p_d[:])
            b1p_sb = constp.tile([128, 2], f32)
            sync_dma(out=b1p_sb[:], in_=b1p_d[:])
            b2p_sb = constp.tile([L, 1], f32)
            sync_dma(out=b2p_sb[:], in_=b2p_d[:])

            # ---- A rows: SBUF-resident table (one-hot lhsT + SBUF-source
            # overflow gather). token tt*128+p -> partition p, rank tt. ----
            a_sb = asbp.tile([128, N_TT, HP2], fp16)
            # ---- G rows: DRAM table, 512B rows ----
            tab = dramp.tile([TC, HP2], fp16)

            # ---- A-side one-hot tiles (DVE; overlapped with stage 1) ----
            s0_tiles = []
            for b in range(NBT):
                s0 = s0p.tile([128, SZ], fp16, tag=f"s0_{b}", name=f"s0_{b}")
                nc.vector.tensor_tensor(
                    out=s0[:],
                    in0=iota_sb[:, 0:1].to_broadcast([128, SZ]),
                    in1=i0v_sb[:, b * SZ:(b + 1) * SZ],
                    op=EQ,
                )
                s0_tiles.append(s0)

            # ---- stage 1: xs @ [W1a | W1g] -> a_sb (SBUF) + tab (DRAM) ----
            tab_writers = []
            last_tab_dma = None
            for tt in range(N_TT):
                ch, hf = tt // 2, tt % 2
                ps = psXp.tile([128, 512], f32, tag="ps")
                for kb in range(N_KB):
                    wch = wc_lo if kb < 4 else wc_hi
                    nc.tensor.matmul(
                        ps[:, 0:W1N],
                        xst_sb[:, ch, kb, hf * 128:(hf + 1) * 128],
                        wch[:, (kb % 4) * W1N:(kb % 4 + 1) * W1N],
                        start=(kb == 0),
                        stop=(kb == N_KB - 1),
                    )
                tab_writers.append(
                    nc.vector.memset(a_sb[:, tt, H:HP2], 0.0)
                )
                tab_writers.append(
                    nc.scalar.activation(a_sb[:, tt, 0:H], ps[:, 0:H], AF.Copy)
                )
                gt = gtp.tile([128, HP2], fp16, tag="gt")
                nc.vector.memset(gt[:, H:HP2], 0.0)
                nc.scalar.activation(gt[:, 0:H], ps[:, H:W1N], AF.Copy)
                # scalar HWDGE ring: independent of the xs-load ring, so
                # table rows land as stage-1 produces them; same-queue FIFO
                # means the last write covers all 16
                last_tab_dma = scalar_dma(
                    out=tab[tt * 128:(tt + 1) * 128, :], in_=gt[:]
                )
                tab_writers.append(last_tab_dma)
            tw_by_name = {w.ins.name: w.ins for w in tab_writers}

            # ---- gather preps: descriptor-gen on Pool, overlapped with the
            # xs load. Emitted AFTER the table writes so the RAW edges are
            # emission-time visible, then demoted to no-sync on the prep
            # (desc-gen doesn't read the table) and re-attached as sync deps
            # of the triggers. ----
            gg_sb = ggp.tile([128, NG, 2, GSZ], fp16)
            ggA = ggp.tile([128, 2, SZ], fp16, tag="ggA", name="ggA")
            gsems = [nc.alloc_semaphore(f"ggs{k}") for k in range(NG + 1)]
            preps = []
            prev_g = None

            def post_prep(g):
                nonlocal prev_g
                for dep in set(g.ins.sync_dependency_names()):
                    if dep in tw_by_name:
                        g.ins.try_remove_dependency(dep)
                        add_dep_helper(g.ins, tw_by_name[dep], False, "defer src")
                if prev_g is not None:
                    add_dep_helper(g.ins, prev_g.ins, False, "pin prep order")
                prev_g = g
                preps.append(g)

            for k in range(NG):
                post_prep(nc.gpsimd.dma_gather(
                    gg_sb[:, k, :, :],
                    tab[:],
                    idxg_sb[:, k * (GSZ // 16):(k + 1) * (GSZ // 16)],
                    GSZ,
                    GSZ,
                    elem_size=HP2,
                    elem_step=HP2,
                    transpose=True,
                    queue_num=k % 4,
                    prepare_only=True,
                    sem=gsems[k],
                ))
            # overflow A rows from the SBUF a-table
            post_prep(nc.gpsimd.dma_gather(
                ggA[:],
                a_sb[:],
                idxa_sb[:],
                SZ,
                SZ,
                elem_size=HP2,
                transpose=True,
                queue_num=NG % 4,
                prepare_only=True,
                sem=gsems[NG],
                sbuf_tokens_per_rank=128,
                sbuf_free_dim_per_rank=2 * HP2,
                sbuf_byte_offset=0,
            ))
            prep_by_name = {p.ins.name: p.ins for p in preps}

            # ---- fire the prepared gathers (table complete) ----
            # One count=1 trigger per gather, in order. Only the first
            # needs the table sync deps -- Pool executes the rest after it.
            for k, p in enumerate(preps):
                q = k % 4 if k < NG else NG % 4
                trig = nc.gpsimd.trigger_dma(count=1, queue_num=q)
                add_dep_helper(trig.ins, p.ins, False, "after prep")
                if k == 0:
                    add_dep_helper(
                        trig.ins, last_tab_dma.ins, True, "tab -> trigger"
                    )
                    for w in tab_writers:
                        if w.ins.name != last_tab_dma.ins.name:
                            add_dep_helper(trig.ins, w.ins, True, "a_sb -> trig")

            def degate(inst, *waits):
                """Swap Tile's prep-write attribution for explicit waits on
                the per-gather DMA-completion sems."""
                for dep in set(inst.ins.sync_dependency_names()):
                    if dep in prep_by_name:
                        inst.ins.try_remove_dependency(dep)
                        add_dep_helper(
                            inst.ins, prep_by_name[dep], False, "manual sem"
                        )
                for s in waits:
                    inst.wait_op(gsems[s], 16, "sem-ge")

            # G data for global span s lives at chunk s//GSZ, col s%GSZ.
            def g_ap(st, lo_hi, c0, c1):
                s = st * SZ + c0
                k, off = s // GSZ, s % GSZ
                assert off + (c1 - c0) <= GSZ
                if lo_hi == 0:
                    return k, gg_sb[:, k, 0, off:off + (c1 - c0)]
                return k, gg_sb[0:22, k, 1, off:off + (c1 - c0)]

            # ---- stage 2 ----
            for st in range(NT):
                # split points where this 512-tile crosses an 896-chunk edge
                lo, hi = st * SZ, (st + 1) * SZ
                cut = -(-lo // GSZ) * GSZ      # next chunk edge >= lo
                segs = [(0, cut - lo), (cut - lo, SZ)] if lo < cut < hi \
                    else [(0, SZ)]
                t0 = t0p.tile([128, SZ], fp16, tag="t0")
                t1 = t1p.tile([22, SZ], fp16, tag="t1")
                if st < NBT:
                    pA0 = psXp.tile([128, SZ], f32, tag="ps")
                    nc.tensor.matmul(
                        pA0[:], a_sb[:, st, 0:128], s0_tiles[st][:],
                        start=True, stop=True,
                    )
                    pA1 = psXp.tile([22, SZ], f32, tag="ps")
                    nc.tensor.matmul(
                        pA1[:], a_sb[:, st, 128:H], s0_tiles[st][:],
                        start=True, stop=True,
                    )
                    for c0, c1 in segs:
                        k, gap = g_ap(st, 0, c0, c1)
                        a0 = nc.vector.tensor_tensor(
                            out=t0[:, c0:c1], in0=pA0[:, c0:c1], in1=gap,
                            op=ADD,
                        )
                        degate(a0, k)
                        k, gap = g_ap(st, 1, c0, c1)
                        a1 = nc.vector.tensor_tensor(
                            out=t1[:, c0:c1], in0=pA1[:, c0:c1], in1=gap,
                            op=ADD,
                        )
                        degate(a1, k)
                else:
                    # overflow tile: A side gathered from SBUF (ggA). The
                    # ggA wait rides a tiny same-engine memset (sync-dep
                    # chained) to stay within the per-inst wait-slot limit.
                    xw = nc.vector.memset(t0[0:1, 0:1], 0.0)
                    xw.wait_op(gsems[NG], 16, "sem-ge")
                    for dep in set(xw.ins.sync_dependency_names()):
                        if dep in prep_by_name:
                            xw.ins.try_remove_dependency(dep)
                    for c0, c1 in segs:
                        k, gap = g_ap(st, 0, c0, c1)
                        a0 = nc.vector.tensor_tensor(
                            out=t0[:, c0:c1], in0=ggA[:, 0, c0:c1], in1=gap,
                            op=ADD,
                        )
                        degate(a0, k)
                        add_dep_helper(a0.ins, xw.ins, True, "ggA wait")
                        k, gap = g_ap(st, 1, c0, c1)
                        a1 = nc.vector.tensor_tensor(
                            out=t1[:, c0:c1], in0=ggA[0:22, 1, c0:c1], in1=gap,
                            op=ADD,
                        )
                        degate(a1, k)
                        add_dep_helper(a1.ins, xw.ins, True, "ggA wait")
                h0 = h0p.tile([128, SZ], fp16, tag="h0")
                h1 = h1p.tile([22, SZ], fp16, tag="h1")
                nc.scalar.activation(h0[:], t0[:], AF.Relu, bias=b1p_sb[:, 0:1])
                nc.scalar.activation(h1[:], t1[:], AF.Relu, bias=b1p_sb[0:22, 1:2])
                ps2 = ps2p.tile([L, SZ], f32, tag="ps2")
                nc.tensor.matmul(
                    ps2[:], w2p_sb[:, 0:L], h0[:], start=True, stop=False
                )
                nc.tensor.matmul(
                    ps2[:], w2p_sb[0:22, L:2 * L], h1[:], start=False, stop=True
                )
                ot = otp.tile([L, SZ], f32)
                if st % 2 == 0:
                    nc.scalar.activation(ot[:], ps2[:], AF.Identity, bias=b2p_sb[:])
                else:
                    nc.vector.tensor_scalar_add(out=ot[:], in0=ps2[:], scalar1=b2p_sb[:])
                sync_dma(
                    out=outT_d[:, st * SZ:(st + 1) * SZ], in_=ot[:]
                )

    # Strip Tile's prep-mode DMASW pre-bump instructions (~1.2us each on
    # the Pool engine, serialized with the preps). After the consumer-wait
    # surgery above the DMASW lane sems have no real waiters: consumers are
    # gated by the per-gather descriptor sems, and the final Drain's DMASW
    # waits are subsumed by its engine-clock waits (every gather's data
    # flows through DVE->ACT->out-DMA, all upstream of the drain). Zero any
    # remaining DMASW waits so nothing blocks on the now-never-bumped sems.
    for b in nc.main_func.blocks:
        il = b.instructions
        for i in [x for x in il if type(x).__name__ == "InstIncSwdgeSem"]:
            il.remove(i)
        for i in il:
            si = i.sync_info
            if si is None:
                continue
            for w in (si.on_wait or []):
                if w.ant_name and w.ant_name.startswith("DMASW"):
                    w.wait_value = 0

    return nc


def _wrap_idx(idx_1d):
    """SWDGE index layout: [16, n/16].T wrapped, replicated to 128 rows."""
    n = len(idx_1d)
    arr16 = idx_1d.reshape(n // 16, 16).T
    return np.tile(arr16, (8, 1)).astype(np.int16)


def prep_inputs(xs, spans, batch_ids, W1, b1, W2, b2):
    """Host-side routing and layout. Returns (in_maps, slot_maps)."""
    xs = np.asarray(xs, dtype=np.float32)
    spans = np.asarray(spans).astype(np.int64)
    batch_ids = np.asarray(batch_ids).astype(np.int64)
    W1 = np.asarray(W1, dtype=np.float32)
    b1 = np.asarray(b1, dtype=np.float32)
    W2 = np.asarray(W2, dtype=np.float32)
    b2 = np.asarray(b2, dtype=np.float32)

    core = batch_ids // BPC
    local0 = (batch_ids % BPC) * T + spans[:, 0]
    local1 = (batch_ids % BPC) * T + spans[:, 1]

    # shared weights
    W1h = W1.astype(np.float16)
    wc = np.empty((128, N_KB * W1N), np.float16)
    for kb in range(N_KB):
        wc[:, kb * W1N:kb * W1N + H] = W1h[kb * 128:(kb + 1) * 128, :]
        wc[:, kb * W1N + H:(kb + 1) * W1N] = W1h[D + kb * 128:D + (kb + 1) * 128, :]
    w2p = np.zeros((128, 2 * L), np.float16)
    w2p[:, 0:L] = W2[0:128].astype(np.float16)
    w2p[0:H - 128, L:2 * L] = W2[128:H].astype(np.float16)
    b1p = np.zeros((128, 2), np.float32)
    b1p[:, 0] = b1[0:128]
    b1p[0:H - 128, 1] = b1[128:H]
    b2p = np.ascontiguousarray(b2.reshape(L, 1))
    iota = np.arange(128, dtype=np.uint8).reshape(128, 1)

    in_maps = []
    slot_maps = []
    for c in range(NCORES):
        sel = np.nonzero(core == c)[0]
        i0 = local0[sel]
        i1 = local1[sel]
        blk = i0 >> 7

        slot_map = np.full(MPAD, -1, np.int64)
        s_i0 = np.zeros(MPAD, np.int64)
        s_i1 = np.zeros(MPAD, np.int64)
        overflow = []
        for b in range(NBT):
            ids = np.nonzero(blk == b)[0]
            # sort by end-token: the G gathers then read the DRAM table
            # mostly in ascending order (better row-buffer locality)
            ids = ids[np.argsort(i1[ids], kind="stable")]
            take = ids[:SZ]
            overflow.append(ids[SZ:])
            base = b * SZ
            ntk = len(take)
            slot_map[base:base + ntk] = sel[take]
            s_i0[base:base + ntk] = i0[take]
            s_i1[base:base + ntk] = i1[take]
            # pad: i0 at block start (one-hot row 0), i1 = 0
            s_i0[base + ntk:base + SZ] = b * 128
        ov = np.concatenate(overflow) if overflow else np.empty(0, np.int64)
        assert len(ov) <= SZ, f"core {c}: overflow {len(ov)} > {SZ}"
        base = NBT * SZ
        nov = len(ov)
        slot_map[base:base + nov] = sel[ov]
        s_i0[base:base + nov] = i0[ov]
        s_i1[base:base + nov] = i1[ov]

        # one-hot compare values: block-relative begin-token, u8
        i0rel = (s_i0[:NBT * SZ] & 127).astype(np.uint8)
        i0v = np.ascontiguousarray(np.broadcast_to(i0rel, (128, NBT * SZ)))
        # G-gather idx stream: MPAD values padded to NG*GSZ, wrapped per chunk
        gidx = np.zeros(GPAD, np.int64)
        gidx[:MPAD] = s_i1
        idxg = np.concatenate(
            [_wrap_idx(gidx[k * GSZ:(k + 1) * GSZ]) for k in range(NG)], axis=1
        )
        idxa = _wrap_idx(s_i0[NBT * SZ:])

        # xsT: [p, ch, kb, t] = xs[token=ch*256+t, d=kb*128+p], fp16
        xs_c = xs[c * BPC:(c + 1) * BPC].reshape(TC, D).astype(np.float16)
        xsT = np.ascontiguousarray(
            xs_c.T.reshape(N_KB, 128, N_CH, 256).transpose(1, 2, 0, 3)
        )

        in_maps.append({
            "xsT": xsT, "wc": wc, "w2p": w2p,
            "b1p": b1p, "b2p": b2p, "iota": iota, "i0v": i0v,
            "idxg": idxg, "idxa": idxa,
        })
        slot_maps.append(slot_map)

    return in_maps, slot_maps


def _scatter_out(results, slot_maps):
    out = np.empty((N_SPANS, L), np.float32)
    for c in range(NCORES):
        sm = slot_maps[c]
        valid = sm >= 0
        out[sm[valid]] = results[c]["outT"].T[valid]
    return out


def _install_ntff_shim():
    """Provide antenv.axon_hooks (missing on this image) so that
    run_bass_kernel_spmd(trace=True) can drive NTFF profiling via the
    axon .so. Only used by the profiling path."""
    import sys
    import types
    import ctypes
    import contextlib

    if "antenv.axon_hooks" in sys.modules:
        return
    import antenv

    holder = {"hook": None}
    mod = types.ModuleType("antenv.axon_hooks")
    mod.set_axon_ntff_profile_hook = lambda h: holder.__setitem__("hook", h)
    mod.get_axon_ntff_profile_hook = lambda: holder["hook"]
    sys.modules["antenv.axon_hooks"] = mod
    antenv.axon_hooks = mod

    so_path = "/opt/axon/libaxon_pjrt.so"
    try:
        lib = ctypes.CDLL(so_path)
    except OSError:
        return
    if not hasattr(lib, "axon_start_nrt_profile"):
        return
    lib.axon_start_nrt_profile.argtypes = [
        ctypes.POINTER(ctypes.c_int64),
        ctypes.c_size_t,
    ]
    lib.axon_start_nrt_profile.restype = ctypes.c_int64
    lib.axon_stop_nrt_profile.argtypes = [ctypes.c_char_p]
    lib.axon_stop_nrt_profile.restype = ctypes.c_int64

    @contextlib.contextmanager
    def _hook(output_dir, device_ids):
        import jax

        jax.devices()
        if device_ids:
            ids = (ctypes.c_int64 * len(device_ids))(*device_ids)
            rc = lib.axon_start_nrt_profile(ids, len(device_ids))
        else:
            rc = lib.axon_start_nrt_profile(None, 0)
        if rc != 0:
            raise RuntimeError(f"axon_start_nrt_profile rc={rc}")
        try:
            yield
        finally:
            n = lib.axon_stop_nrt_profile(str(output_dir).encode())
            print(f"profile: {n} file(s) written to {output_dir}")

    mod.set_axon_ntff_profile_hook(_hook)


def run(inputs: dict, trace: bool = False):
    """Run on the 8 NeuronCores. Returns (out, BassKernelResults)."""
    from concourse import bass_utils
    from concourse.bass_utils import run_bass_kernel_spmd

    if trace:
        _install_ntff_shim()
        bass_utils.upload_artifacts = lambda tmpdir: str(tmpdir)

    in_maps, slot_maps = prep_inputs(**inputs)
    nc = build_graph()
    nc.finalize()
    res = run_bass_kernel_spmd(
        nc, in_maps, list(range(NCORES)), trace=trace
    )
    return _scatter_out(res.results, slot_maps), res


def kernel(**inputs) -> np.ndarray:
    out, _ = run(inputs, trace=False)
    return out


# revision 26
# speedup vs baseline: 1.4269x; 1.0090x over previous
"""Trainium2 Bass kernel for nn_BaselineSpanScorer (span-pair MLP scorer), v6.

reference:
    xs        [32, 512, 1024] f32
    spans     [65536, 2] int   (begin/end token index within sequence)
    batch_ids [65536] int
    W1 [2048, 150], b1 [150], W2 [150, 17], b2 [17]
    out[n] = relu(concat(xs[b, s0], xs[b, s1]) @ W1 + b1) @ W2 + b2

Strategy (8 NeuronCores, data parallel, per-core graph is input-shape fixed):
  - Shard xs by batch: core c owns batches [4c, 4c+4) = 2048 token rows.
  - Factorize: A[t] = xs[t] @ W1[:1024], G[t] = xs[t] @ W1[1024:], so
    pre[n] = A[i0_n] + G[i1_n] + b1 (token reuse cuts matmul work ~4x).
  - Stage 1: 16 token tiles x 8 K-blocks of fused [A|G] 300-col fp16
    matmuls. A rows stay in SBUF (one-hot lhsT + SBUF-source overflow
    gather); G rows go to a 1MB DRAM table (512B rows).
  - Spans are bucketed by i0 token block (16 buckets of 512 + 1 overflow
    tile, MPAD=8704): a block tile's one-hot contraction needs exactly
    ONE 128-token window. Overflow spans gather A from SBUF.
  - G side via 10 DRAM-source transpose dma_gathers of 896 idxs each
    (s2m ring descriptors 114 <= 128, the real per-op limit; fewer, fatter
    gathers cut the ~2us/op Pool-engine descriptor-gen serial cost).
    All 11 gathers are PREPARED during the xs load and fired by per-gather
    trigger_dma(count=1) once the table lands. Tile's prep-mode DMASW
    plumbing is bypassed: pre-bump insts stripped post-build, consumers
    wait on per-gather descriptor-completion sems instead.
  - Stage 2 per 512-span tile: PSUM accumulates onehot-A (or SBUF-gathered
    A); gathered G added on DVE (split in two where a 512-tile crosses an
    896-gather-chunk boundary); relu+bias on ACT; [150]x[17] W2
    contraction; bias; DMA out scores^T.
  - Host scatters per-core outputs back to original span order.

Compute dtype fp16 (rel err ~1e-3 vs f32 reference), f32 output.
"""

import os

os.environ.setdefault("MYCRO_LOCAL_CACHE", "1")

import numpy as np

# ---------------- problem constants (hardcoded per spec) ----------------
B, T, D = 32, 512, 1024
N_SPANS = 65536
H, L = 150, 17
NCORES = 8
BPC = B // NCORES        # batches per core = 4
TC = BPC * T             # tokens per core = 2048
N_KB = D // 128          # K blocks in stage 1 = 8
N_TT = TC // 128         # token tiles in stage 1 = 16
N_CH = 8                 # xsT load chunks (2 token tiles each)
SZ = 512                 # spans per stage-2 tile
NBT = TC // 128          # block tiles = 16
NT = NBT + 1             # + 1 overflow tile
MPAD = NT * SZ           # padded span count per core = 8704
HP2 = 256                # table row elems (fp16 -> 512B)
W1N = 2 * H              # 300: stage-1 moving operand width (A | G)
GSZ = 896                # G-gather idx count (s2m = 896/16*2+2 = 114 <= 128)
NG = -(-MPAD // GSZ)     # G-gather chunks = 10
GPAD = NG * GSZ          # padded G-idx stream = 8960


def build_graph():
    """Build the per-core SPMD Bass graph (fixed shapes, input independent)."""
    from concourse import bacc
    import concourse.mybir as mybir
    from concourse.tile import TileContext
    from concourse import library_config
    from concourse.tile_rust import add_dep_helper

    fp16 = mybir.dt.float16
    f32 = mybir.dt.float32
    i16 = mybir.dt.int16
    u8 = mybir.dt.uint8
    AF = mybir.ActivationFunctionType
    EQ = mybir.AluOpType.is_equal
    ADD = mybir.AluOpType.add

    nc = bacc.Bacc(num_swdge_queues=4)

    _chain = []
    _schain = []

    def sync_dma(**kw):
        g = nc.sync.dma_start(**kw)
        if _chain:
            add_dep_helper(g.ins, _chain[-1].ins, False, "pin dma order")
        _chain.append(g)
        return g

    def scalar_dma(**kw):
        g = nc.scalar.dma_start(**kw)
        if _schain:
            add_dep_helper(g.ins, _schain[-1].ins, False, "pin dma order")
        _schain.append(g)
        return g

    xsT_d = nc.declare_dram_parameter("xsT", [128, N_CH, N_KB, 256], fp16, isOutput=False)
    wc_d = nc.declare_dram_parameter("wc", [128, N_KB * W1N], fp16, isOutput=False)
    w2p_d = nc.declare_dram_parameter("w2p", [128, 2 * L], fp16, isOutput=False)
    b1p_d = nc.declare_dram_parameter("b1p", [128, 2], f32, isOutput=False)
    b2p_d = nc.declare_dram_parameter("b2p", [L, 1], f32, isOutput=False)
    iota_d = nc.declare_dram_parameter("iota", [128, 1], u8, isOutput=False)
    i0v_d = nc.declare_dram_parameter("i0v", [128, NBT * SZ], u8, isOutput=False)
    idxg_d = nc.declare_dram_parameter("idxg", [128, NG * GSZ // 16], i16, isOutput=False)
    idxa_d = nc.declare_dram_parameter("idxa", [128, 32], i16, isOutput=False)
    outT_d = nc.declare_dram_parameter("outT", [L, MPAD], f32, isOutput=True)

    with TileContext(nc) as tc:
        with (
            tc.tile_pool(name="const", bufs=1) as constp,
            tc.tile_pool(name="xst", bufs=1) as xstp,
            tc.tile_pool(name="asb", bufs=1) as asbp,
            tc.tile_pool(name="gt", bufs=16) as gtp,
            tc.tile_pool(name="dram", bufs=1, space="DRAM") as dramp,
            tc.tile_pool(name="s0", bufs=1) as s0p,
            tc.tile_pool(name="gg", bufs=1) as ggp,
            tc.tile_pool(name="psX", bufs=5, space="PSUM") as psXp,
            tc.tile_pool(name="ps2", bufs=3, space="PSUM") as ps2p,
            tc.tile_pool(name="h0", bufs=6) as h0p,
            tc.tile_pool(name="t0", bufs=6) as t0p,
            tc.tile_pool(name="t1", bufs=6) as t1p,
            tc.tile_pool(name="h1", bufs=6) as h1p,
            tc.tile_pool(name="ot", bufs=8) as otp,
        ):
            nc.gpsimd.load_library(library_config.mlp)

            # ---- input loads ----
            # stage-1 weights first (PE needs them at t~12us), then the
            # small gather-index tensors (Pool-engine desc-gen needs them
            # early), then the bulk xs^T chunks + one-hot compare rows.
            wc_lo = constp.tile([128, 4 * W1N], fp16)
            wc_hi = constp.tile([128, 4 * W1N], fp16)
            wc_dma = sync_dma(out=wc_lo[:], in_=wc_d[:, 0:4 * W1N])
            idxg_sb = constp.tile([128, NG * GSZ // 16], i16)
            sync_dma(out=idxg_sb[:], in_=idxg_d[:])
            idxa_sb = constp.tile([128, 32], i16)
            sync_dma(out=idxa_sb[:], in_=idxa_d[:])
            iota_sb = constp.tile([128, 1], u8)
            sync_dma(out=iota_sb[:], in_=iota_d[:])
            sync_dma(out=wc_hi[:], in_=wc_d[:, 4 * W1N:])
            xst_sb = xstp.tile([128, N_CH, N_KB, 256], fp16)
            i0v_sb = constp.tile([128, NBT * SZ], u8)
            for ch in range(N_CH):
                sync_dma(out=xst_sb[:, ch, :, :], in_=xsT_d[:, ch, :, :])
            # one-hot compare rows after the bulk load: DVE IS_EQ output is
            # first consumed by stage-2 matmuls, well after the load
            sync_dma(out=i0v_sb[:], in_=i0v_d[:])
            w2p_sb = constp.tile([128, 2 * L], fp16)
            sync_dma(out=w2p_sb[:], in_=w2p_d[:])
            b1p_sb = constp.tile([128, 2], f32)
            sync_dma(out=b1p_sb[:], in_=b1p_d[:])
            b2p_sb = constp.tile([L, 1], f32)
            sync_dma(out=b2p_sb[:], in_=b2p_d[:])

            # ---- A rows: SBUF-resident table (one-hot lhsT + SBUF-source
            # overflow gather). token tt*128+p -> partition p, rank tt. ----
            a_sb = asbp.tile([128, N_TT, HP2], fp16)
            # ---- G rows: DRAM table, 512B rows ----
            tab = dramp.tile([TC, HP2], fp16)

            # ---- A-side one-hot tiles (DVE; overlapped with stage 1) ----
            s0_tiles = []
            for b in range(NBT):
                s0 = s0p.tile([128, SZ], fp16, tag=f"s0_{b}", name=f"s0_{b}")
                nc.vector.tensor_tensor(
                    out=s0[:],
                    in0=iota_sb[:, 0:1].to_broadcast([128, SZ]),
                    in1=i0v_sb[:, b * SZ:(b + 1) * SZ],
                    op=EQ,
                )
                s0_tiles.append(s0)

            # ---- stage 1: xs @ [W1a | W1g] -> a_sb (SBUF) + tab (DRAM) ----
            tab_writers = []
            last_tab_dma = None
            for tt in range(N_TT):
                ch, hf = tt // 2, tt % 2
                ps = psXp.tile([128, 512], f32, tag="ps")
                for kb in range(N_KB):
                    wch = wc_lo if kb < 4 else wc_hi
                    nc.tensor.matmul(
                        ps[:, 0:W1N],
                        xst_sb[:, ch, kb, hf * 128:(hf + 1) * 128],
                        wch[:, (kb % 4) * W1N:(kb % 4 + 1) * W1N],
                        start=(kb == 0),
                        stop=(kb == N_KB - 1),
                    )
                tab_writers.append(
                    nc.vector.memset(a_sb[:, tt, H:HP2], 0.0)
                )
                tab_writers.append(
                    nc.scalar.activation(a_sb[:, tt, 0:H], ps[:, 0:H], AF.Copy)
                )
                gt = gtp.tile([128, HP2], fp16, tag="gt")
                nc.vector.memset(gt[:, H:HP2], 0.0)
                nc.scalar.activation(gt[:, 0:H], ps[:, H:W1N], AF.Copy)
                # scalar HWDGE ring: independent of the xs-load ring, so
                # table rows land as stage-1 produces them; same-queue FIFO
                # means the last write covers all 16
                last_tab_dma = scalar_dma(
                    out=tab[tt * 128:(tt + 1) * 128, :], in_=gt[:]
                )
                tab_writers.append(last_tab_dma)
            tw_by_name = {w.ins.name: w.ins for w in tab_writers}

            # ---- gather preps: descriptor-gen on Pool, overlapped with the
            # xs load. Emitted AFTER the table writes so the RAW edges are
            # emission-time visible, then demoted to no-sync on the prep
            # (desc-gen doesn't read the table) and re-attached as sync deps
            # of the triggers. ----
            gg_sb = ggp.tile([128, NG, 2, GSZ], fp16)
            ggA = ggp.tile([128, 2, SZ], fp16, tag="ggA", name="ggA")
            gsems = [nc.alloc_semaphore(f"ggs{k}") for k in range(NG + 1)]
            preps = []
            prev_g = None

            def post_prep(g):
                nonlocal prev_g
                for dep in set(g.ins.sync_dependency_names()):
                    if dep in tw_by_name:
                        g.ins.try_remove_dependency(dep)
                        add_dep_helper(g.ins, tw_by_name[dep], False, "defer src")
                if prev_g is not None:
                    add_dep_helper(g.ins, prev_g.ins, False, "pin prep order")
                prev_g = g
                preps.append(g)

            for k in range(NG):
                post_prep(nc.gpsimd.dma_gather(
                    gg_sb[:, k, :, :],
                    tab[:],
                    idxg_sb[:, k * (GSZ // 16):(k + 1) * (GSZ // 16)],
                    GSZ,
                    GSZ,
                    elem_size=HP2,
                    elem_step=HP2,
                    transpose=True,
                    queue_num=k % 4,
                    prepare_only=True,
                    sem=gsems[k],
                ))
            # overflow A rows from the SBUF a-table
            post_prep(nc.gpsimd.dma_gather(
                ggA[:],
                a_sb[:],
                idxa_sb[:],
                SZ,
                SZ,
                elem_size=HP2,
                transpose=True,
                queue_num=NG % 4,
                prepare_only=True,
                sem=gsems[NG],
                sbuf_tokens_per_rank=128,
                sbuf_free_dim_per_rank=2 * HP2,
                sbuf_byte_offset=0,
            ))
            prep_by_name = {p.ins.name: p.ins for p in preps}

            # ---- fire the prepared gathers (table complete) ----
            # One count=1 trigger per gather, in order. Only the first
            # needs the table sync deps -- Pool executes the rest after it.
            for k, p in enumerate(preps):
                q = k % 4 if k < NG else NG % 4
                trig = nc.gpsimd.trigger_dma(count=1, queue_num=q)
                add_dep_helper(trig.ins, p.ins, False, "after prep")
                if k == 0:
                    add_dep_helper(
                        trig.ins, last_tab_dma.ins, True, "tab -> trigger"
                    )
                    for w in tab_writers:
                        if w.ins.name != last_tab_dma.ins.name:
                            add_dep_helper(trig.ins, w.ins, True, "a_sb -> trig")

            def degate(inst, *waits):
                """Swap Tile's prep-write attribution for explicit waits on
                the per-gather DMA-completion sems."""
                for dep in set(inst.ins.sync_dependency_names()):
                    if dep in prep_by_name:
                        inst.ins.try_remove_dependency(dep)
                        add_dep_helper(
                            inst.ins, prep_by_name[dep], False, "manual sem"
                        )
                for s in waits:
                    inst.wait_op(gsems[s], 16, "sem-ge")

            # G data for global span s lives at chunk s//GSZ, col s%GSZ.
            def g_ap(st, lo_hi, c0, c1):
                s = st * SZ + c0
                k, off = s // GSZ, s % GSZ
                assert off + (c1 - c0) <= GSZ
                if lo_hi == 0:
                    return k, gg_sb[:, k, 0, off:off + (c1 - c0)]
                return k, gg_sb[0:22, k, 1, off:off + (c1 - c0)]

            # ---- stage 2 ----
            # One-hot matmuls are emitted one tile AHEAD of the consuming
            # ADD/RELU/W2 chain: they depend only on stage-1 outputs, so
            # the PE always has runnable work queued instead of idling a
            # DVE->ACT roundtrip per tile (which also resets its p-state
            # ramp, halving the clock).
            pA_tiles = {}

            def emit_onehots(st):
                if st >= NBT or st in pA_tiles:
                    return
                pA0 = psXp.tile([128, SZ], f32, tag="ps")
                nc.tensor.matmul(
                    pA0[:], a_sb[:, st, 0:128], s0_tiles[st][:],
                    start=True, stop=True,
                )
                pA1 = psXp.tile([22, SZ], f32, tag="ps")
                nc.tensor.matmul(
                    pA1[:], a_sb[:, st, 128:H], s0_tiles[st][:],
                    start=True, stop=True,
                )
                pA_tiles[st] = (pA0, pA1)

            for st in range(NT):
                emit_onehots(st)
                emit_onehots(st + 1)
                # split points where this 512-tile crosses an 896-chunk edge
                lo, hi = st * SZ, (st + 1) * SZ
                cut = -(-lo // GSZ) * GSZ      # next chunk edge >= lo
                segs = [(0, cut - lo), (cut - lo, SZ)] if lo < cut < hi \
                    else [(0, SZ)]
                t0 = t0p.tile([128, SZ], fp16, tag="t0")
                t1 = t1p.tile([22, SZ], fp16, tag="t1")
                if st < NBT:
                    pA0, pA1 = pA_tiles.pop(st)
                    for c0, c1 in segs:
                        k, gap = g_ap(st, 0, c0, c1)
                        a0 = nc.vector.tensor_tensor(
                            out=t0[:, c0:c1], in0=pA0[:, c0:c1], in1=gap,
                            op=ADD,
                        )
                        degate(a0, k)
                        k, gap = g_ap(st, 1, c0, c1)
                        a1 = nc.vector.tensor_tensor(
                            out=t1[:, c0:c1], in0=pA1[:, c0:c1], in1=gap,
                            op=ADD,
                        )
                        degate(a1, k)
                else:
                    # overflow tile: A side gathered from SBUF (ggA). The
                    # ggA wait rides a tiny same-engine memset (sync-dep
                    # chained) to stay within the per-inst wait-slot limit.
                    xw = nc.vector.memset(t0[0:1, 0:1], 0.0)
                    xw.wait_op(gsems[NG], 16, "sem-ge")
                    for dep in set(xw.ins.sync_dependency_names()):
                        if dep in prep_by_name:
                            xw.ins.try_remove_dependency(dep)
                    for c0, c1 in segs:
                        k, gap = g_ap(st, 0, c0, c1)
                        a0 = nc.vector.tensor_tensor(
                            out=t0[:, c0:c1], in0=ggA[:, 0, c0:c1], in1=gap,
                            op=ADD,
                        )
                        degate(a0, k)
                        add_dep_helper(a0.ins, xw.ins, True, "ggA wait")
                        k, gap = g_ap(st, 1, c0, c1)
                        a1 = nc.vector.tensor_tensor(
                            out=t1[:, c0:c1], in0=ggA[0:22, 1, c0:c1], in1=gap,
                            op=ADD,
                        )
                        degate(a1, k)
                        add_dep_helper(a1.ins, xw.ins, True, "ggA wait")
                h0 = h0p.tile([128, SZ], fp16, tag="h0")
                h1 = h1p.tile([22, SZ], fp16, tag="h1")
                nc.scalar.activation(h0[:], t0[:], AF.Relu, bias=b1p_sb[:, 0:1])
                nc.scalar.activation(h1[:], t1[:], AF.Relu, bias=b1p_sb[0:22, 1:2])
                ps2 = ps2p.tile([L, SZ], f32, tag="ps2")
                nc.tensor.matmul(
                    ps2[:], w2p_sb[:, 0:L], h0[:], start=True, stop=False
                )
                nc.tensor.matmul(
                    ps2[:], w2p_sb[0:22, L:2 * L], h1[:], start=False, stop=True
                )
                ot = otp.tile([L, SZ], f32)
                if st % 2 == 0:
                    nc.scalar.activation(ot[:], ps2[:], AF.Identity, bias=b2p_sb[:])
                else:
                    nc.vector.tensor_scalar_add(out=ot[:], in0=ps2[:], scalar1=b2p_sb[:])
                sync_dma(
                    out=outT_d[:, st * SZ:(st + 1) * SZ], in_=ot[:]
                )

    # Strip Tile's prep-mode DMASW pre-bump instructions (~1.2us each on
    # the Pool engine, serialized with the preps). After the consumer-wait
    # surgery above the DMASW lane sems have no real waiters: consumers are
    # gated by the per-gather descriptor sems, and the final Drain's DMASW
    # waits are subsumed by its engine-clock waits (every gather's data
    # flows through DVE->ACT->out-DMA, all upstream of the drain). Zero any
    # remaining DMASW waits so nothing blocks on the now-never-bumped sems.
    for b in nc.main_func.blocks:
        il = b.instructions
        for i in [x for x in il if type(x).__name__ == "InstIncSwdgeSem"]:
            il.remove(i)
        for i in il:
            si = i.sync_info
            if si is None:
                continue
            for w in (si.on_wait or []):
                if w.ant_name and w.ant_name.startswith("DMASW"):
                    w.wait_value = 0

    return nc


def _wrap_idx(idx_1d):
    """SWDGE index layout: [16, n/16].T wrapped, replicated to 128 rows."""
    n = len(idx_1d)
    arr16 = idx_1d.reshape(n // 16, 16).T
    return np.tile(arr16, (8, 1)).astype(np.int16)


def prep_inputs(xs, spans, batch_ids, W1, b1, W2, b2):
    """Host-side routing and layout. Returns (in_maps, slot_maps)."""
    xs = np.asarray(xs, dtype=np.float32)
    spans = np.asarray(spans).astype(np.int64)
    batch_ids = np.asarray(batch_ids).astype(np.int64)
    W1 = np.asarray(W1, dtype=np.float32)
    b1 = np.asarray(b1, dtype=np.float32)
    W2 = np.asarray(W2, dtype=np.float32)
    b2 = np.asarray(b2, dtype=np.float32)

    core = batch_ids // BPC
    local0 = (batch_ids % BPC) * T + spans[:, 0]
    local1 = (batch_ids % BPC) * T + spans[:, 1]

    # shared weights
    W1h = W1.astype(np.float16)
    wc = np.empty((128, N_KB * W1N), np.float16)
    for kb in range(N_KB):
        wc[:, kb * W1N:kb * W1N + H] = W1h[kb * 128:(kb + 1) * 128, :]
        wc[:, kb * W1N + H:(kb + 1) * W1N] = W1h[D + kb * 128:D + (kb + 1) * 128, :]
    w2p = np.zeros((128, 2 * L), np.float16)
    w2p[:, 0:L] = W2[0:128].astype(np.float16)
    w2p[0:H - 128, L:2 * L] = W2[128:H].astype(np.float16)
    b1p = np.zeros((128, 2), np.float32)
    b1p[:, 0] = b1[0:128]
    b1p[0:H - 128, 1] = b1[128:H]
    b2p = np.ascontiguousarray(b2.reshape(L, 1))
    iota = np.arange(128, dtype=np.uint8).reshape(128, 1)

    in_maps = []
    slot_maps = []
    for c in range(NCORES):
        sel = np.nonzero(core == c)[0]
        i0 = local0[sel]
        i1 = local1[sel]
        blk = i0 >> 7

        slot_map = np.full(MPAD, -1, np.int64)
        s_i0 = np.zeros(MPAD, np.int64)
        s_i1 = np.zeros(MPAD, np.int64)
        overflow = []
        for b in range(NBT):
            ids = np.nonzero(blk == b)[0]
            # sort by end-token: the G gathers then read the DRAM table
            # mostly in ascending order (better row-buffer locality)
            ids = ids[np.argsort(i1[ids], kind="stable")]
            take = ids[:SZ]
            overflow.append(ids[SZ:])
            base = b * SZ
            ntk = len(take)
            slot_map[base:base + ntk] = sel[take]
            s_i0[base:base + ntk] = i0[take]
            s_i1[base:base + ntk] = i1[take]
            # pad: i0 at block start (one-hot row 0), i1 = 0
            s_i0[base + ntk:base + SZ] = b * 128
        ov = np.concatenate(overflow) if overflow else np.empty(0, np.int64)
        assert len(ov) <= SZ, f"core {c}: overflow {len(ov)} > {SZ}"
        base = NBT * SZ
        nov = len(ov)
        slot_map[base:base + nov] = sel[ov]
        s_i0[base:base + nov] = i0[ov]
        s_i1[base:base + nov] = i1[ov]

        # one-hot compare values: block-relative begin-token, u8
        i0rel = (s_i0[:NBT * SZ] & 127).astype(np.uint8)
        i0v = np.ascontiguousarray(np.broadcast_to(i0rel, (128, NBT * SZ)))
        # G-gather idx stream: MPAD values padded to NG*GSZ, wrapped per chunk
        gidx = np.zeros(GPAD, np.int64)
        gidx[:MPAD] = s_i1
        idxg = np.concatenate(
            [_wrap_idx(gidx[k * GSZ:(k + 1) * GSZ]) for k in range(NG)], axis=1
        )
        idxa = _wrap_idx(s_i0[NBT * SZ:])

        # xsT: [p, ch, kb, t] = xs[token=ch*256+t, d=kb*128+p], fp16
        xs_c = xs[c * BPC:(c + 1) * BPC].reshape(TC, D).astype(np.float16)
        xsT = np.ascontiguousarray(
            xs_c.T.reshape(N_KB, 128, N_CH, 256).transpose(1, 2, 0, 3)
        )

        in_maps.append({
            "xsT": xsT, "wc": wc, "w2p": w2p,
            "b1p": b1p, "b2p": b2p, "iota": iota, "i0v": i0v,
            "idxg": idxg, "idxa": idxa,
        })
        slot_maps.append(slot_map)

    return in_maps, slot_maps


def _scatter_out(results, slot_maps):
    out = np.empty((N_SPANS, L), np.float32)
    for c in range(NCORES):
        sm = slot_maps[c]
        valid = sm >= 0
        out[sm[valid]] = results[c]["outT"].T[valid]
    return out


def _install_ntff_shim():
    """Provide antenv.axon_hooks (missing on this image) so that
    run_bass_kernel_spmd(trace=True) can drive NTFF profiling via the
    axon .so. Only used by the profiling path."""
    import sys
    import types
    import ctypes
    import contextlib

    if "antenv.axon_hooks" in sys.modules:
        return
    import antenv

    holder = {"hook": None}
    mod = types.ModuleType("antenv.axon_hooks")
    mod.set_axon_ntff_profile_hook = lambda h: holder.__setitem__("hook", h)
    mod.get_axon_ntff_profile_hook = lambda: holder["hook"]
    sys.modules["antenv.axon_hooks"] = mod
    antenv.axon_hooks = mod

    so_path = "/opt/axon/libaxon_pjrt.so"
    try:
        lib = ctypes.CDLL(so_path)
    except OSError:
        return
    if not hasattr(lib, "axon_start_nrt_profile"):
        return
    lib.axon_start_nrt_profile.argtypes = [
        ctypes.POINTER(ctypes.c_int64),
        ctypes.c_size_t,
    ]
    lib.axon_start_nrt_profile.restype = ctypes.c_int64
    lib.axon_stop_nrt_profile.argtypes = [ctypes.c_char_p]
    lib.axon_stop_nrt_profile.restype = ctypes.c_int64

    @contextlib.contextmanager
    def _hook(output_dir, device_ids):
        import jax

        jax.devices()
        if device_ids:
            ids = (ctypes.c_int64 * len(device_ids))(*device_ids)
            rc = lib.axon_start_nrt_profile(ids, len(device_ids))
        else:
            rc = lib.axon_start_nrt_profile(None, 0)
        if rc != 0:
            raise RuntimeError(f"axon_start_nrt_profile rc={rc}")
        try:
            yield
        finally:
            n = lib.axon_stop_nrt_profile(str(output_dir).encode())
            print(f"profile: {n} file(s) written to {output_dir}")

    mod.set_axon_ntff_profile_hook(_hook)


def run(inputs: dict, trace: bool = False):
    """Run on the 8 NeuronCores. Returns (out, BassKernelResults)."""
    from concourse import bass_utils
    from concourse.bass_utils import run_bass_kernel_spmd

    if trace:
        _install_ntff_shim()
        bass_utils.upload_artifacts = lambda tmpdir: str(tmpdir)

    in_maps, slot_maps = prep_inputs(**inputs)
    nc = build_graph()
    nc.finalize()
    res = run_bass_kernel_spmd(
        nc, in_maps, list(range(NCORES)), trace=trace
    )
    return _scatter_out(res.results, slot_maps), res


def kernel(**inputs) -> np.ndarray:
    out, _ = run(inputs, trace=False)
    return out
